# revision 65
# baseline (speedup 1.0000x reference)
"""Trainium2 Bass kernel for nn_Attention_47545287967487.

Causal multi-head attention (B=2, S=2048, D=1024, H=16, DH=64) with QK
RMS-norm, distributed over 8 NeuronCores.

Distribution: head tensor-parallel for the QKV projections and attention
(each core owns 2 heads = a 128-column slice of Wq/Wk/Wv, computing the
full 4096-row sequence), then ONE AllToAll per iteration redistributes
the bf16 attention outputs of both batches so each core owns a 256-row
slice per batch and runs the output projection locally with the full
Wo. This moves ~1MB/core over the fabric instead of ReduceScattering a
16MB fp32 partial, the final out write is 2MB instead of 16MB, and a
single merged exchange pays the collective launch cost (~5-6us
measured on HW) once instead of twice.

Numerics: x/W/QK^T/PV run in bf16 (fp32 PSUM accumulation), softmax in
fp32->bf16. Scores are bounded (|q.k|/8 <= 8 after RMS-norm) so softmax
skips the max-subtraction pass; a constant -2.25 bias inside the exp is
harmless (it cancels in the normalization). An fp8e4m3 P/V variant with
DoubleRow PV was tried and rejected: it measured rel err 2.7e-2 against
the 2e-2 gate (P/V quantization ~4% each).

Engine plan per core:
 - PE: x@W projections (bf16, K-tiled), QK^T with the two heads packed
   into array row-groups (tile_position), PV as [v|1]^T @ P so the
   softmax denominator is a free 65th output row, the local output
   projection, selector matmuls for the q+k sum-of-squares rows and the
   rstd broadcast, and 128x128 transposes of v into [t,d] layout.
 - ACT: one exp per (chunk, key-tile) covering BOTH heads via a
   [128,2,npx] access pattern over a 2-bank PSUM pair tile; ONE
   Square / Ln / Exp per chunk for the q+k rstd chain (q and k side by
   side on partitions 0:2 of a [2,1024] tile). The 1/sqrt(DH) score
   scale folds into the q normalize multiply.
 - DVE: PSUM->SBUF casts/copies on the critical projection chain, q/k
   normalize multiplies, causal mask multiplies (only the 128-col
   diagonal block), and softmax denominator reciprocals.
 - Pool/GPSIMD: the AllToAll collective, denominator partition
   broadcasts, and the attention-output normalize multiplies (these
   read only SBUF — GPSIMD cannot touch PSUM — and have a full
   iteration of slack before their a_dr-store consumers, so they
   tolerate Pool stalls and decongest DVE, whose queue otherwise
   delays the projection normalize chain).

Scheduling: one FLAT stream of steps across all repeats (so repeat
tails overlap the next repeat's head) that software-pipelines at three
levels. Chunk c's projection parts (~8) are WOVEN between the j-tiles
of chunk c-2's attention, paced evenly, so the PE consumes them while
ACT runs the serial exp chain and the normalize chain's DVE/ACT
latency is hidden by a full extra step; within a chunk the QK matmul
runs two j-tiles AHEAD of the exp (the 3-slot score-pair budget's max)
and the PV matmul one behind it, so the PE never sits on an exp even
while the pipeline refills at chunk starts; and the output-projection parts weave in two and three steps
after the AllToAll so the in-order PE queue never head-of-line blocks
on the exchange. The raw q|k projection is copied off PSUM immediately
(qkr), releasing its pair slot in ~1.2us and letting the whole
normalize chain run in bf16.

PSUM discipline: exactly 8 banks = one pool with a [128,1024] fp32
"pair" tag (bufs=3; holds q+k projection pairs, v projections + v
transposes, the sumsq/broadcast scratch, score pairs, and output-
projection pairs) plus two [65,512] attention accumulators.

kernel(**inputs) takes the FULL unsharded inputs and returns the FULL
[2, 2048, 1024] float32 output.
"""

import numpy as np

import concourse.bacc as bacc
import concourse.mybir as mybir
from concourse import tile
from concourse.bass_utils import run_bass_kernel_spmd

import ml_dtypes

BF16 = ml_dtypes.bfloat16

# Problem shape (hardcoded per the harness contract).
B, S, D, DH = 2, 2048, 1024, 64
H = D // DH
N_CORES = 8
HEADS_PER_CORE = H // N_CORES          # 2
DC = HEADS_PER_CORE * DH               # 128 feature columns per core
EPS = 1e-6

SCHUNK = 512                            # s-chunk width
TT = 128                                # t-tile width
KT = D // 128                           # 8 contraction tiles
NCH = S // SCHUNK                       # 4 s-chunks per batch
ROWS = B * S                            # 4096
RPB = S // N_CORES                      # 256 output rows per core per batch
EXP_BIAS = -2.25                        # softmax headroom shift (cancels)
QSCALE = 1.0 / (DH ** 0.5)              # folded into the q normalize

F32 = mybir.dt.float32
F32R = mybir.dt.float32r
BF = mybir.dt.bfloat16

# All ACT functions this kernel uses (Square, Ln, Exp, Copy) live in the
# 'natural_log_exp_and_others' table. The default table chooser picks the
# first table containing each function, which thrashes between the exp and
# ln tables (~1.3us per reload, dozens of reloads). Pin the chooser to the
# one table that covers everything by emptying the others (positions are
# preserved so act_func_set_id still indexes act_info.json correctly).
_PINNED_ACT_TABLE = "natural_log_exp_and_others"
_orig_get_act_tables = bacc.get_activation_tables


def _pinned_act_tables(arch):
    tables = _orig_get_act_tables(arch)
    return {
        name: (funcs if name == _PINNED_ACT_TABLE else set())
        for name, funcs in tables.items()
    }


bacc.get_activation_tables = _pinned_act_tables


def build_nc(collective=True, stage=3, repeat=1, abl=()):
    # abl: timing-only ablation flags ("xdma", "exp", "norm", "attnorm",
    # "opmm") — skip instruction classes to attribute backend time. Output
    # is garbage when any flag is set; used by ablate.py only.
    abl = frozenset(abl)
    nc = bacc.Bacc("TRN2", target_bir_lowering=False)

    xt_d = nc.dram_tensor("xt", [D, ROWS], BF, kind="ExternalInput")
    # weights are host-transposed to the SBUF layout [128, KT*cols]: free
    # block k holds W rows [128k, 128k+128), so the DMA is a straight copy
    # with 2KB+ contiguous lines (256B lines pay a 2x DMA latency penalty).
    wq_d = nc.dram_tensor("wq", [128, KT * DC], BF, kind="ExternalInput")
    wk_d = nc.dram_tensor("wk", [128, KT * DC], BF, kind="ExternalInput")
    wv_d = nc.dram_tensor("wv", [128, KT * DC], BF, kind="ExternalInput")
    wo_d = nc.dram_tensor("wo", [128, KT * D], BF, kind="ExternalInput")
    negtri_d = nc.dram_tensor("negtri", [128, 128], BF, kind="ExternalInput")
    ident_d = nc.dram_tensor("ident", [128, 128], BF, kind="ExternalInput")
    selb_d = nc.dram_tensor("selb", [128, 128], BF, kind="ExternalInput")
    selbq_d = nc.dram_tensor("selbq", [128, 128], BF, kind="ExternalInput")
    out_d = nc.dram_tensor("out", [2 * RPB, D], F32, kind="ExternalOutput")

    from contextlib import ExitStack
    with tile.TileContext(nc) as tc:
        with ExitStack() as ctx:
            consts = ctx.enter_context(tc.tile_pool(name="consts", bufs=1))
            wpool = ctx.enter_context(tc.tile_pool(name="wpool", bufs=1))
            persist = ctx.enter_context(tc.tile_pool(name="persist", bufs=1))
            xcp = ctx.enter_context(tc.tile_pool(name="xc", bufs=3))
            sqp = ctx.enter_context(tc.tile_pool(name="sqp", bufs=2))
            stdp = ctx.enter_context(tc.tile_pool(name="stdp", bufs=2))
            bcp = ctx.enter_context(tc.tile_pool(name="bcp", bufs=2))
            vtp = ctx.enter_context(tc.tile_pool(name="vtp", bufs=2))
            vaugp = ctx.enter_context(tc.tile_pool(name="vaugp", bufs=10))
            pp = ctx.enter_context(tc.tile_pool(name="pp", bufs=8))
            attsbp = ctx.enter_context(tc.tile_pool(name="attsb", bufs=2))
            zbp = ctx.enter_context(tc.tile_pool(name="zbp", bufs=4))
            rcp = ctx.enter_context(tc.tile_pool(name="rcp", bufs=4))
            attallp = ctx.enter_context(tc.tile_pool(name="attall", bufs=3))
            gsp = ctx.enter_context(tc.tile_pool(name="gsp", bufs=2))
            outsbp = ctx.enter_context(tc.tile_pool(name="outsb", bufs=2))
            ps = ctx.enter_context(tc.tile_pool(name="ps", bufs=1, space="PSUM"))
            dram = ctx.enter_context(tc.tile_pool(name="dram", bufs=1, space="DRAM"))

            # ---- DMA issue order gates the pipeline head: wq first, then
            # the first x chunk (so the first projection parts can start
            # ~2us in), then wk/wv/consts. All transfers serialize on the
            # shared DMA-engine pool, so issue order == arrival order. ----
            w_sb = {}
            w_tiles = {}
            for wname, wd in (("q", wq_d), ("k", wk_d), ("v", wv_d)):
                t = wpool.tile([128, KT * DC], BF, name=f"w{wname}")
                w_tiles[wname] = (t, wd)
                for k in range(KT):
                    w_sb[(wname, k)] = t[:, k * DC:(k + 1) * DC]

            def load_w(wname):
                t, wd = w_tiles[wname]
                nc.sync.dma_start(t[:], wd[:])

            load_w("q")

            selb_sb = consts.tile([128, 128], BF, name="selb_sb")
            selbq_sb = consts.tile([128, 128], BF, name="selbq_sb")
            ident_sb = consts.tile([128, 128], BF, name="ident_sb")
            negtri_sb = consts.tile([128, 128], BF, name="negtri_sb")

            def load_consts():
                nc.sync.dma_start(selb_sb[:], selb_d[:])
                nc.sync.dma_start(selbq_sb[:], selbq_d[:])
                nc.sync.dma_start(ident_sb[:], ident_d[:])
                nc.sync.dma_start(negtri_sb[:], negtri_d[:])

            eps_sb = consts.tile([128, 1], F32, name="eps_sb")
            nc.vector.memset(eps_sb[:], EPS)
            zero_sb = consts.tile([128, 1], F32, name="zero_sb")
            nc.vector.memset(zero_sb[:], 0.0)
            ebias_sb = consts.tile([128, 1], F32, name="ebias_sb")
            nc.vector.memset(ebias_sb[:], EXP_BIAS)
            # wo is only needed by the first output projection (~half-way in);
            # its 2MB DMA is deferred into the pipeline so it doesn't delay
            # the first x-chunk prefetches behind it in the queue.
            wo_sb = wpool.tile([128, KT * D], BF, name="wo_sb")
            wo_loaded = [False]

            def load_wo():
                if not wo_loaded[0]:
                    wo_loaded[0] = True
                    nc.sync.dma_start(wo_sb[:], wo_d[:])

            # DRAM staging for ONE AllToAll per BATCH (batch 0's exchange +
            # output projection then hide under batch 1's attention).
            # Row-block j = [my 128 features for row-range j of batch b];
            # after the AllToAll, block s = core s's features for THIS
            # core's row ranges of batch b.
            a_dr = [dram.tile([N_CORES * DC, RPB], BF, name=f"a_dr{b}")
                    for b in range(B)]
            if collective:
                g_dr = [dram.tile([N_CORES * DC, RPB], BF, name=f"g_dr{b}")
                        for b in range(B)]
            else:
                g_dr = a_dr  # collective-free variant for TimelineSim

            # per-chunk normalized q|k bf16, feature-major: one [128, 1024]
            # tile per chunk, q*rstd/sqrt(DH) at cols [0:512], k*rstd at
            # [512:1024] (QSCALE folds into the q-half sumsq selector).
            qkns = {}   # (b, i) -> [DC, 2*SCHUNK] bf16
            vaug = {}   # (b, j) -> [128, 65] bf16 slice: [v|1] per head
            pqks = {}   # (b, i) -> in-flight q|k projection PSUM pair

            xcs = {}
            rep_box = [0]

            def prefetch_x(b, i):
                rep = rep_box[0]
                col0 = b * S + i * SCHUNK
                xc = xcp.tile([128, KT * SCHUNK], BF, name=f"x_{rep}_{b}_{i}",
                              tag="xc")
                # one DMA per k-tile: early projection parts can start
                # before the rest of the chunk lands
                if "xdma" not in abl:
                    for k in range(KT):
                        nc.sync.dma_start(
                            xc[:, k * SCHUNK:(k + 1) * SCHUNK],
                            xt_d[k * 128:(k + 1) * 128, col0:col0 + SCHUNK])
                else:
                    nc.vector.memset(xc[:, 0:8], 0.0)
                xcs[(b, i)] = xc

            def proj_qk_mm(b, i, xch, half, ks):
                rep = rep_box[0]
                if half == 0 and ks[0] == 0:
                    pqks[(b, i)] = ps.tile(
                        [128, 2 * SCHUNK], F32, name=f"pqk_{rep}_{b}_{i}",
                        tag="pair", bufs=2)
                pqk = pqks[(b, i)]
                wname = "qk"[half]
                for k in ks:
                    nc.tensor.matmul(
                        pqk[:, half * SCHUNK:(half + 1) * SCHUNK],
                        w_sb[(wname, k)][:], xch[k][:], start=(k == 0),
                        stop=(k == KT - 1))

            # The normalize chain is split into THREE weave parts so its PE
            # matmuls (ssbc sumsq, rstd broadcast) are issued a few j-tiles
            # AFTER their DVE/ACT producers and never head-of-line block the
            # in-order PE queue: norm_a (DVE: qkr copy + square), norm_b1
            # (PE sumsq + ACT Ln/Exp), norm_b2 (PE broadcast + DVE muls).
            norm_st = {}

            def proj_qk_norm_a(b, i):
                rep = rep_box[0]
                # raw q|k off PSUM in ONE copy (frees the pair slot); the
                # normalize chain then runs in bf16.
                qkr = sqp.tile([128, 2 * SCHUNK], BF,
                               name=f"qkr_{rep}_{b}_{i}", tag="qkr")
                sq = sqp.tile([128, 2 * SCHUNK], BF,
                              name=f"sq_{rep}_{b}_{i}", tag="sq")
                norm_st[(b, i)] = (qkr, sq)
                pqk = pqks.pop((b, i))
                if "norm" in abl:
                    nc.vector.memset(qkr[:, 0:8], 0.0)
                    nc.vector.memset(sq[:, 0:8], 0.0)
                    return
                nc.vector.tensor_copy(qkr[:], pqk[:])
                # square on DVE (bf16 2x), not ACT: keeps the sumsq matmul's
                # dependency off the exp-saturated ACT queue
                nc.vector.tensor_mul(sq[:], qkr[:], qkr[:])

            def proj_qk_norm_b1(b, i):
                if "norm" in abl:
                    return
                rep = rep_box[0]
                qkr, sq = norm_st[(b, i)]
                # selb has 1/DH in each head's 64x64 diagonal block, so ONE
                # matmul per half yields the per-head MEAN of squares already
                # broadcast over the head's 64 rows (output rows cost the
                # same regardless of partition count). The q half uses selbq
                # (entries 1.0 = 1/DH * DH) so its rstd comes out scaled by
                # 1/sqrt(DH): QSCALE folds in for free and q and k normalize
                # in ONE multiply downstream.
                ssbc = ps.tile([128, 2 * SCHUNK], F32, name=f"ssbc_{rep}_{b}_{i}",
                               tag="pair", bufs=2)
                for half, sel in ((0, selbq_sb), (1, selb_sb)):
                    nc.tensor.matmul(
                        ssbc[:, half * SCHUNK:(half + 1) * SCHUNK], sel[:],
                        sq[:, half * SCHUNK:(half + 1) * SCHUNK],
                        start=True, stop=True)
                lm = stdp.tile([128, 2 * SCHUNK], F32, name=f"lm_{rep}_{b}_{i}",
                               tag="lm")
                nc.scalar.activation(lm[:], ssbc[:],
                                     mybir.ActivationFunctionType.Ln,
                                     bias=eps_sb[:])
                rstd = bcp.tile([128, 2 * SCHUNK], BF, name=f"rstd_{rep}_{b}_{i}",
                                tag="rstd")
                nc.scalar.activation(rstd[:], lm[:],
                                     mybir.ActivationFunctionType.Exp,
                                     scale=-0.5, bias=zero_sb[:])
                norm_st[(b, i)] = (qkr, rstd)

            def proj_qk_norm_b2(b, i):
                rep = rep_box[0]
                qkn = persist.tile([DC, 2 * SCHUNK], BF, name=f"qkn_{rep}_{b}_{i}",
                                   tag="qkn", bufs=8)
                qkns[(b, i)] = qkn
                if "norm" in abl:
                    nc.vector.memset(qkn[:, 0:8], 0.0)
                    return
                qkr, rstd = norm_st.pop((b, i))
                nc.vector.tensor_mul(qkn[:], qkr[:], rstd[:])

            def proj_v_mm(b, i, xch, ks):
                rep = rep_box[0]
                if ks[0] == 0:
                    pqks[(b, i, "v")] = ps.tile(
                        [128, 2 * SCHUNK], F32, name=f"pv_{rep}_{b}_{i}",
                        tag="pair", bufs=2)
                psv = pqks[(b, i, "v")]
                for k in ks:
                    nc.tensor.matmul(psv[:, 0:SCHUNK], w_sb[("v", k)][:],
                                     xch[k][:], start=(k == 0),
                                     stop=(k == KT - 1))

            def proj_v_fin(b, i):
                rep = rep_box[0]
                psv = pqks.pop((b, i, "v"))
                vt = vtp.tile([DC, SCHUNK], BF, name=f"vt_{rep}_{b}_{i}",
                              tag="vt")
                nc.vector.tensor_copy(vt[:], psv[:, 0:SCHUNK])
                # transposes reuse the (dead) second bank of the psv slot —
                # their outputs land CONTIGUOUSLY so one DVE copy + one
                # memset assembles all four [v|1] j-tiles of the chunk.
                # (A DMA-xbar transpose was tried instead: ~1.9us init per
                # transfer in the backend's cost model made it far worse.)
                for u in range(SCHUNK // TT):
                    tpb = psv[:, SCHUNK + 64 * u:SCHUNK + 64 * (u + 1)].bitcast(BF)
                    nc.tensor.transpose(tpb[:], vt[:, u * 128:(u + 1) * 128],
                                        ident_sb[:])
                va4 = vaugp.tile([128, 4 * 2 * (DH + 1)], BF,
                                 name=f"va_{rep}_{b}_{i}", tag="vaug")
                va4v = va4[:].rearrange("p (u g d) -> p u g d", u=4, g=2)
                nc.vector.tensor_copy(
                    va4v[:, :, :, 0:DH],
                    psv[:, SCHUNK:SCHUNK + 4 * 64].bitcast(BF)[:]
                    .rearrange("p (u g d) -> p u g d", u=4, g=2))
                nc.vector.memset(va4v[:, :, :, DH:DH + 1], 1.0)
                for u in range(SCHUNK // TT):
                    j = i * (SCHUNK // TT) + u
                    vaug[(b, j)] = va4[:, u * 2 * (DH + 1):(u + 1) * 2 * (DH + 1)]

            def proj_parts(b, i):
                xc = xcs.pop((b, i))
                xch = [xc[:, k * SCHUNK:(k + 1) * SCHUNK] for k in range(KT)]
                # (pe_cost, fn) — cost in 512-col matmul units, used to pace
                # the weave by PE work rather than by part count
                return [
                    (2, lambda: proj_qk_mm(b, i, xch, 0, [0, 1])),
                    (2, lambda: proj_qk_mm(b, i, xch, 0, [2, 3])),
                    (2, lambda: proj_qk_mm(b, i, xch, 0, [4, 5])),
                    (2, lambda: proj_qk_mm(b, i, xch, 0, [6, 7])),
                    (2, lambda: proj_qk_mm(b, i, xch, 1, [0, 1])),
                    (2, lambda: proj_qk_mm(b, i, xch, 1, [2, 3])),
                    (2, lambda: proj_qk_mm(b, i, xch, 1, [4, 5])),
                    (2, lambda: proj_qk_mm(b, i, xch, 1, [6, 7])),
                    (0, lambda: proj_qk_norm_a(b, i)),
                    (2, lambda: proj_v_mm(b, i, xch, [0, 1])),
                    (2, lambda: proj_v_mm(b, i, xch, [2, 3])),
                    (2, lambda: proj_qk_norm_b1(b, i)),
                    (2, lambda: proj_v_mm(b, i, xch, [4, 5])),
                    (2, lambda: proj_v_mm(b, i, xch, [6, 7])),
                    (0, lambda: proj_qk_norm_b2(b, i)),
                    (1, lambda: proj_v_fin(b, i)),
                ]

            def do_proj(b, i):
                for _, part in proj_parts(b, i):
                    part()

            def do_attn(b, i, weave=None):
                rep = rep_box[0]
                # TWO [65, 1024] accumulators (even/odd j-tiles), head h at
                # cols [512h, 512h+512): halves the serial PV-accumulate
                # dependency chain; a DVE add merges them into asb (same op
                # count as the old staging copy). Chunk 0 is all-diagonal
                # (offj > 0 from j=1) so it keeps a single accumulator.
                n_acc = 1 if i == 0 else 2
                atts = [ps.tile([DH + 1, 2 * SCHUNK], F32,
                                name=f"att_{rep}_{b}_{i}_{e}", tag=f"att{e}",
                                bufs=1)
                        for e in range(n_acc)]
                n_t = 4 * i + 4
                parts = list(weave) if weave else []
                wtot = sum(c for c, _ in parts) or 1
                wdone = [0.0]
                psbs = {}

                def pv(j):
                    # PV for tile j, one software-pipeline stage behind the
                    # exp so the PE never waits on the current tile's exp
                    offj = max(0, TT * (j - 4 * i))
                    pj = psbs.pop(j)
                    att = atts[j % n_acc]
                    for h in range(HEADS_PER_CORE):
                        nc.tensor.matmul(
                            att[:, SCHUNK * h + offj:SCHUNK * (h + 1)],
                            vaug[(b, j)][:, h * (DH + 1):(h + 1) * (DH + 1)],
                            pj[:, SCHUNK * h + offj:SCHUNK * (h + 1)],
                            start=(j < n_acc), stop=(j >= n_t - n_acc),
                        )

                pts = {}

                def qk(j):
                    off = max(0, TT * (j - 4 * i))
                    jc, ju = j // 4, j % 4
                    diag = j >= 4 * i
                    # both heads' scores in one 2-bank pair tile: head h at
                    # cols [512h+off, 512h+512)
                    pt = ps.tile([128, 2 * SCHUNK], F32,
                                 name=f"ptile_{rep}_{b}_{i}_{j}", tag="pair", bufs=2)
                    pts[j] = pt
                    qkn_i = qkns[(b, i)]
                    qkn_j = qkns[(b, jc)]
                    for h in range(HEADS_PER_CORE):
                        nc.tensor.matmul(
                            pt[:, SCHUNK * h + off:SCHUNK * (h + 1)],
                            qkn_j[h * DH:(h + 1) * DH,
                                  SCHUNK + ju * TT:SCHUNK + (ju + 1) * TT],
                            qkn_i[h * DH:(h + 1) * DH, off:SCHUNK],
                            start=True, stop=not diag,
                            tile_position=(h * DH, 0),
                        )
                        if diag:
                            # causal mask as a PE accumulate: -1000 on the
                            # strictly-upper triangle of the diagonal block
                            # (exp underflows to exactly 0); frees the DVE
                            # of per-tile mask multiplies
                            nc.tensor.matmul(
                                pt[:, SCHUNK * h + off:SCHUNK * h + off + TT],
                                negtri_sb[:], ident_sb[:],
                                start=False, stop=True)

                def expmask(j):
                    off = max(0, TT * (j - 4 * i))
                    pt = pts.pop(j)
                    # one exp covers both heads via the [128, 2, npx] view
                    psb = pp.tile([128, 2 * SCHUNK], BF,
                                  name=f"p_{rep}_{b}_{i}_{j}", tag="p")
                    psbs[j] = psb
                    if "exp" in abl:
                        nc.vector.memset(psb[:, 0:8], 0.0)
                        return
                    ptv = pt[:].rearrange("p (h c) -> p h c", h=2)
                    psv = psb[:].rearrange("p (h c) -> p h c", h=2)
                    nc.scalar.activation(
                        psv[:, :, off:SCHUNK], ptv[:, :, off:SCHUNK],
                        mybir.ActivationFunctionType.Exp, bias=ebias_sb[:])

                # deepened inner pipeline: QK(j) runs one tile ahead of
                # exp(j-1) and two ahead of PV(j-2), so the PE never sits on
                # the first exp at a chunk start while the pipeline refills
                for j in range(n_t):
                    qk(j)
                    if j >= 1:
                        expmask(j - 1)
                    # weave projection/output-projection work here, paced by
                    # cumulative PE cost across j-tiles so the PE stays fed
                    # while ACT runs the exp chain
                    want = wtot * (j + 1) / n_t
                    while parts and wdone[0] < want:
                        c, fn = parts.pop(0)
                        wdone[0] += max(c, 0.25)
                        fn()
                    if j >= 3:
                        pv(j - 3)
                expmask(n_t - 1)
                for j in range(max(0, n_t - 3), n_t):
                    pv(j)

                for _, part in parts:
                    part()

                # ---- normalize by softmax denominator ----
                # at_all64 is [64 v-dims, (h, q)]-major; the a_dr store DMA
                # scatters (h, d) back into the 128-feature row blocks.
                at_all = attallp.tile([DH, 2 * SCHUNK], BF,
                                      name=f"atall_{rep}_{b}_{i}", tag="attall")
                if "attnorm" in abl:
                    nc.vector.memset(at_all[:, 0:8], 0.0)
                else:
                    # merge the even/odd accumulators off PSUM in one DVE op
                    # (releases the att banks for the next chunk); the rest
                    # of the normalize runs on Pool/GPSIMD mid-pipeline to
                    # decongest DVE, but on DVE for the batch's last chunk,
                    # whose chain gates the AllToAll.
                    asb = attsbp.tile([DH + 1, 2 * SCHUNK], F32,
                                      name=f"asb_{rep}_{b}_{i}", tag="asb")
                    nc.vector.tensor_copy(asb[:], atts[0][:])
                    if n_acc == 2:
                        nc.vector.tensor_add(asb[:], asb[:], atts[1][:])
                    rc = rcp.tile([1, 2 * SCHUNK], F32,
                                  name=f"rc_{rep}_{b}_{i}", tag="rc")
                    nc.vector.reciprocal(rc[:], asb[DH:DH + 1, :])
                    zbs = zbp.tile([DH, 2 * SCHUNK], F32,
                                   name=f"zbs_{rep}_{b}_{i}", tag="zb")
                    nc.gpsimd.partition_broadcast(zbs[:], rc[:])
                    if i == NCH - 1:
                        nc.vector.tensor_mul(at_all[:], asb[0:DH, :], zbs[:])
                    else:
                        nc.gpsimd.tensor_mul(at_all[:], asb[0:DH, :], zbs[:])

                # stage this chunk's attention output for its batch's
                # AllToAll: chunk i covers row-ranges 2i and 2i+1;
                # a_dr row (2i+half)*128 + 64h + d <- at_all64[d, 512h +
                # 256*half + c] (one store per half keeps the APs 3-D)
                av = at_all[:].rearrange("d (h half c) -> d h half c",
                                         h=2, half=2)
                for half in range(2):
                    r0 = (2 * i + half) * DC
                    nc.sync.dma_start(
                        a_dr[b][r0:r0 + DC, :]
                        .rearrange("(h d) c -> d h c", h=2),
                        av[:, :, half, :])

            def do_a2a(b):
                if collective:
                    nc.gpsimd.collective_compute(
                        "AllToAll",
                        mybir.AluOpType.bypass,
                        replica_groups=[list(range(N_CORES))],
                        ins=[a_dr[b][:]],
                        outs=[g_dr[b][:]],
                    )

            def outproj_parts(b):
                rep = rep_box[0]
                # load gathered A^T [1024 feats, 256 rows] as k-tile blocks
                gsb = gsp.tile([128, KT * RPB], BF, name=f"gsb_{rep}_{b}",
                               tag="gsb")
                # two half-loads so the first matmuls (k-tiles 0-3) start a
                # DMA earlier at the tail
                hk = KT // 2
                for u in range(2):
                    nc.sync.dma_start(
                        gsb[:, u * hk * RPB:(u + 1) * hk * RPB]
                        .rearrange("p (k c) -> p k c", k=hk),
                        g_dr[b][u * hk * 128:(u + 1) * hk * 128, :]
                        .rearrange("(k p) c -> p k c", p=128))
                ops = {}

                def mm(rt, n, ks):
                    if n == 0 and ks[0] == 0:
                        ops[rt] = ps.tile([128, 2 * SCHUNK], F32,
                                          name=f"op_{rep}_{b}_{rt}",
                                          tag="pair", bufs=2)
                    if "opmm" in abl:
                        if n == 0 and ks[0] == 0:
                            nc.vector.memset(ops[rt][:, 0:8], 0.0)
                        return
                    for k in ks:
                        nc.tensor.matmul(
                            ops[rt][:, n * SCHUNK:(n + 1) * SCHUNK],
                            gsb[:, RPB * k + TT * rt:RPB * k + TT * (rt + 1)],
                            wo_sb[:, D * k + SCHUNK * n:
                                  D * k + SCHUNK * (n + 1)],
                            start=(k == 0), stop=(k == KT - 1))

                def fin(rt):
                    # (PSUM->DRAM DMA is not supported; stage through SBUF)
                    osb = outsbp.tile([128, D], F32, name=f"osb_{rep}_{b}_{rt}",
                                      tag="osb")
                    nc.vector.tensor_copy(osb[:], ops.pop(rt)[:])
                    nc.sync.dma_start(
                        out_d[(2 * b + rt) * TT:(2 * b + rt + 1) * TT, :],
                        osb[:])

                parts = []
                for rt in range(2):
                    for n in range(2):
                        parts.append((4, lambda rt=rt, n=n: mm(rt, n, [0, 1, 2, 3])))
                        parts.append((4, lambda rt=rt, n=n: mm(rt, n, [4, 5, 6, 7])))
                    parts.append((0, lambda rt=rt: fin(rt)))
                return parts

            # Software pipeline, one FLAT stream of steps across all repeats
            # (no rep-boundary special cases). Step s: attention for global
            # chunk s-1, WOVEN with the projection parts for chunk s+1 (two
            # steps of slack for the normalize chain's DVE/ACT latency) and
            # any output-projection parts scheduled for this step; x for
            # chunk s+2 is prefetched. A batch's AllToAll is issued as soon
            # as its last chunk's attention has staged its output, and its
            # output projection weaves in two steps later so the in-order PE
            # queue never head-of-line blocks on the exchange.
            chunks = [(b, i) for b in range(B) for i in range(NCH)]
            nch = len(chunks)
            total = repeat * nch
            pending = {}
            pf = [0]

            def pf_to(limit):
                while pf[0] < min(limit, total):
                    rep_box[0] = pf[0] // nch
                    prefetch_x(*chunks[pf[0] % nch])
                    pf[0] += 1

            # preamble DMA order: wq (issued at tile creation above) ->
            # x chunk 0 -> wk, wv -> consts; the first projection can then
            # start as soon as wq + the first x k-tiles land (~2us).
            pf_to(1)
            load_w("k")
            load_w("v")
            load_consts()

            for s in range(total + 1):
                pf_to(s + 3)
                if s == 1:
                    load_wo()
                weave = []
                if s + 1 < total:
                    rep_box[0] = (s + 1) // nch
                    weave += proj_parts(*chunks[(s + 1) % nch])
                weave += pending.pop(s, [])
                if s >= 1 and stage >= 2:
                    g = s - 1
                    rep_box[0] = g // nch
                    b_prev, i_prev = chunks[g % nch]
                    do_attn(b_prev, i_prev, weave=weave)
                    if stage >= 3 and i_prev == NCH - 1:
                        # this batch's last chunk is staged: exchange now;
                        # its output projection weaves in two steps later
                        # (batch 0's entirely overlaps batch 1's attention)
                        do_a2a(b_prev)
                        pending[s + 2] = outproj_parts(b_prev)
                else:
                    rep_box[0] = 0
                    do_proj(*chunks[0])
                    for _, part in weave:
                        part()
            for parts in pending.values():
                for _, part in parts:
                    part()

    nc.compile()
    return nc


_NC_CACHE = {}


def _get_nc():
    if "nc" not in _NC_CACHE:
        _NC_CACHE["nc"] = build_nc()
    return _NC_CACHE["nc"]


def _ktile_layout(w):
    # [K*128, C] -> [128, K*C]: free block k holds rows [128k, 128k+128)
    kt, c = w.shape[0] // 128, w.shape[1]
    return np.ascontiguousarray(
        w.reshape(kt, 128, c).transpose(1, 0, 2).reshape(128, kt * c))


def _host_inputs(x, Wq, Wk, Wv, Wo):
    xt = np.ascontiguousarray(
        np.asarray(x, dtype=np.float32).reshape(ROWS, D).T).astype(BF16)
    ident = np.eye(128, dtype=BF16)
    # causal-mask accumulate: out[m,n] += negtri[n,m] = -1000 iff key m >
    # query n (strictly upper triangle); exp then underflows to exactly 0
    negtri = np.triu(np.full((128, 128), -1000.0, dtype=np.float32), 1).astype(BF16)
    # block-diagonal mean-of-squares selector: selb[d, m] = 1/DH when d and
    # m fall in the same 64-row head group (output = per-head mean already
    # broadcast over the head's rows); selbq additionally folds the
    # 1/sqrt(DH) score scale into the q-half rstd (entries 1.0 = DH/DH)
    selb = np.zeros((128, 128), dtype=np.float32)
    for h in range(HEADS_PER_CORE):
        selb[h * DH:(h + 1) * DH, h * DH:(h + 1) * DH] = 1.0 / DH
    selbq = (selb * DH).astype(BF16)
    selb = selb.astype(BF16)
    wo_h = _ktile_layout(np.asarray(Wo, dtype=np.float32)).astype(BF16)

    in_maps = []
    for c in range(N_CORES):
        cs = c * DC
        in_maps.append({
            "xt": xt,
            "wq": _ktile_layout(np.asarray(Wq, dtype=np.float32)[:, cs:cs + DC]).astype(BF16),
            "wk": _ktile_layout(np.asarray(Wk, dtype=np.float32)[:, cs:cs + DC]).astype(BF16),
            "wv": _ktile_layout(np.asarray(Wv, dtype=np.float32)[:, cs:cs + DC]).astype(BF16),
            "wo": wo_h,
            "negtri": negtri,
            "ident": ident,
            "selb": selb,
            "selbq": selbq,
        })
    return in_maps


def kernel(x, Wq, Wk, Wv, Wo, mask):
    x = np.asarray(x, dtype=np.float32)
    nc = _get_nc()
    in_maps = _host_inputs(x, np.asarray(Wq), np.asarray(Wk),
                           np.asarray(Wv), np.asarray(Wo))
    res = run_bass_kernel_spmd(nc, in_maps, list(range(N_CORES)))
    full = np.empty((ROWS, D), dtype=np.float32)
    for c in range(N_CORES):
        o = res.results[c]["out"]
        for b in range(B):
            r0 = b * S + c * RPB
            full[r0:r0 + RPB] = o[b * RPB:(b + 1) * RPB]
    return full.reshape(B, S, D)


if __name__ == "__main__":
    nc = build_nc()
    print("kernel built and compiled OK")



# revision 69
# speedup vs baseline: 1.1529x; 1.1529x over previous
"""Trainium2 Bass kernel for nn_Attention_47545287967487.

Causal multi-head attention (B=2, S=2048, D=1024, H=16, DH=64) with QK
RMS-norm, distributed over 8 NeuronCores.

Distribution: head tensor-parallel for the QKV projections and attention
(each core owns 2 heads = a 128-column slice of Wq/Wk/Wv, computing the
full 4096-row sequence), then ONE AllToAll per iteration redistributes
the bf16 attention outputs of both batches so each core owns a 256-row
slice per batch and runs the output projection locally with the full
Wo. This moves ~1MB/core over the fabric instead of ReduceScattering a
16MB fp32 partial, the final out write is 2MB instead of 16MB, and a
single merged exchange pays the collective launch cost (~5-6us
measured on HW) once instead of twice.

Numerics: x/W/QK^T/PV run in bf16 (fp32 PSUM accumulation), softmax in
fp32->bf16. Scores are bounded (|q.k|/8 <= 8 after RMS-norm) so softmax
skips the max-subtraction pass; a constant -2.25 bias inside the exp is
harmless (it cancels in the normalization). An fp8e4m3 P/V variant with
DoubleRow PV was tried and rejected: it measured rel err 2.7e-2 against
the 2e-2 gate (P/V quantization ~4% each).

Engine plan per core:
 - PE: x@W projections (bf16, K-tiled), QK^T with the two heads packed
   into array row-groups (tile_position), PV as [v|1]^T @ P so the
   softmax denominator is a free 65th output row, the local output
   projection, selector matmuls for the q+k sum-of-squares rows and the
   rstd broadcast, and 128x128 transposes of v into [t,d] layout.
 - ACT: one exp per (chunk, key-tile) covering BOTH heads via a
   [128,2,npx] access pattern over a 2-bank PSUM pair tile; ONE
   Square / Ln / Exp per chunk for the q+k rstd chain (q and k side by
   side on partitions 0:2 of a [2,1024] tile). The 1/sqrt(DH) score
   scale folds into the q normalize multiply.
 - DVE: PSUM->SBUF casts/copies on the critical projection chain, q/k
   normalize multiplies, causal mask multiplies (only the 128-col
   diagonal block), and softmax denominator reciprocals.
 - Pool/GPSIMD: the AllToAll collective, denominator partition
   broadcasts, and the attention-output normalize multiplies (these
   read only SBUF — GPSIMD cannot touch PSUM — and have a full
   iteration of slack before their a_dr-store consumers, so they
   tolerate Pool stalls and decongest DVE, whose queue otherwise
   delays the projection normalize chain).

Scheduling: one FLAT stream of steps across all repeats (so repeat
tails overlap the next repeat's head) that software-pipelines at three
levels. Chunk c's projection parts (~8) are WOVEN between the j-tiles
of chunk c-2's attention, paced evenly, so the PE consumes them while
ACT runs the serial exp chain and the normalize chain's DVE/ACT
latency is hidden by a full extra step; within a chunk the QK matmul
runs two j-tiles AHEAD of the exp (the 3-slot score-pair budget's max)
and the PV matmul one behind it, so the PE never sits on an exp even
while the pipeline refills at chunk starts; and the output-projection parts weave in two and three steps
after the AllToAll so the in-order PE queue never head-of-line blocks
on the exchange. The raw q|k projection is copied off PSUM immediately
(qkr), releasing its pair slot in ~1.2us and letting the whole
normalize chain run in bf16.

PSUM discipline: exactly 8 banks = one pool with a [128,1024] fp32
"pair" tag (bufs=3; holds q+k projection pairs, v projections + v
transposes, the sumsq/broadcast scratch, score pairs, and output-
projection pairs) plus two [65,512] attention accumulators.

kernel(**inputs) takes the FULL unsharded inputs and returns the FULL
[2, 2048, 1024] float32 output.
"""

import numpy as np

import concourse.bacc as bacc
import concourse.mybir as mybir
from concourse import tile
from concourse.bass_utils import run_bass_kernel_spmd

import ml_dtypes

BF16 = ml_dtypes.bfloat16

# Problem shape (hardcoded per the harness contract).
B, S, D, DH = 2, 2048, 1024, 64
H = D // DH
N_CORES = 8
HEADS_PER_CORE = H // N_CORES          # 2
DC = HEADS_PER_CORE * DH               # 128 feature columns per core
EPS = 1e-6

SCHUNK = 512                            # s-chunk width
TT = 128                                # t-tile width
KT = D // 128                           # 8 contraction tiles
NCH = S // SCHUNK                       # 4 s-chunks per batch
ROWS = B * S                            # 4096
RPB = S // N_CORES                      # 256 output rows per core per batch
EXP_BIAS = -2.25                        # softmax headroom shift (cancels)
QSCALE = 1.0 / (DH ** 0.5)              # folded into the q normalize

F32 = mybir.dt.float32
F32R = mybir.dt.float32r
BF = mybir.dt.bfloat16

# All ACT functions this kernel uses (Square, Ln, Exp, Copy) live in the
# 'natural_log_exp_and_others' table. The default table chooser picks the
# first table containing each function, which thrashes between the exp and
# ln tables (~1.3us per reload, dozens of reloads). Pin the chooser to the
# one table that covers everything by emptying the others (positions are
# preserved so act_func_set_id still indexes act_info.json correctly).
_PINNED_ACT_TABLE = "natural_log_exp_and_others"
_orig_get_act_tables = bacc.get_activation_tables


def _pinned_act_tables(arch):
    tables = _orig_get_act_tables(arch)
    return {
        name: (funcs if name == _PINNED_ACT_TABLE else set())
        for name, funcs in tables.items()
    }


bacc.get_activation_tables = _pinned_act_tables


def build_nc(collective=True, stage=3, repeat=1, abl=()):
    # abl: timing-only ablation flags ("xdma", "exp", "norm", "attnorm",
    # "opmm") — skip instruction classes to attribute backend time. Output
    # is garbage when any flag is set; used by ablate.py only.
    abl = frozenset(abl)
    nc = bacc.Bacc("TRN2", target_bir_lowering=False)

    xt_d = nc.dram_tensor("xt", [D, ROWS], BF, kind="ExternalInput")
    # weights are host-transposed to the SBUF layout [128, KT*cols]: free
    # block k holds W rows [128k, 128k+128), so the DMA is a straight copy
    # with 2KB+ contiguous lines (256B lines pay a 2x DMA latency penalty).
    wq_d = nc.dram_tensor("wq", [128, KT * DC], BF, kind="ExternalInput")
    wk_d = nc.dram_tensor("wk", [128, KT * DC], BF, kind="ExternalInput")
    wv_d = nc.dram_tensor("wv", [128, KT * DC], BF, kind="ExternalInput")
    wo_d = nc.dram_tensor("wo", [128, KT * D], BF, kind="ExternalInput")
    negtri_d = nc.dram_tensor("negtri", [128, 128], BF, kind="ExternalInput")
    ident_d = nc.dram_tensor("ident", [128, 128], BF, kind="ExternalInput")
    selb_d = nc.dram_tensor("selb", [128, 128], BF, kind="ExternalInput")
    selbq_d = nc.dram_tensor("selbq", [128, 128], BF, kind="ExternalInput")
    out_d = nc.dram_tensor("out", [2 * RPB, D], F32, kind="ExternalOutput")

    from contextlib import ExitStack
    with tile.TileContext(nc) as tc:
        with ExitStack() as ctx:
            consts = ctx.enter_context(tc.tile_pool(name="consts", bufs=1))
            wpool = ctx.enter_context(tc.tile_pool(name="wpool", bufs=1))
            persist = ctx.enter_context(tc.tile_pool(name="persist", bufs=1))
            xcp = ctx.enter_context(tc.tile_pool(name="xc", bufs=3))
            sqp = ctx.enter_context(tc.tile_pool(name="sqp", bufs=2))
            stdp = ctx.enter_context(tc.tile_pool(name="stdp", bufs=2))
            bcp = ctx.enter_context(tc.tile_pool(name="bcp", bufs=2))
            vtp = ctx.enter_context(tc.tile_pool(name="vtp", bufs=2))
            vaugp = ctx.enter_context(tc.tile_pool(name="vaugp", bufs=10))
            pp = ctx.enter_context(tc.tile_pool(name="pp", bufs=8))
            attsbp = ctx.enter_context(tc.tile_pool(name="attsb", bufs=2))
            zbp = ctx.enter_context(tc.tile_pool(name="zbp", bufs=4))
            rcp = ctx.enter_context(tc.tile_pool(name="rcp", bufs=4))
            attallp = ctx.enter_context(tc.tile_pool(name="attall", bufs=3))
            gsp = ctx.enter_context(tc.tile_pool(name="gsp", bufs=2))
            outsbp = ctx.enter_context(tc.tile_pool(name="outsb", bufs=2))
            ps = ctx.enter_context(tc.tile_pool(name="ps", bufs=1, space="PSUM"))
            dram = ctx.enter_context(tc.tile_pool(name="dram", bufs=1, space="DRAM"))

            # ---- DMA issue order gates the pipeline head: wq first, then
            # the first x chunk (so the first projection parts can start
            # ~2us in), then wk/wv/consts. All transfers serialize on the
            # shared DMA-engine pool, so issue order == arrival order. ----
            w_sb = {}
            w_tiles = {}
            for wname, wd in (("q", wq_d), ("k", wk_d), ("v", wv_d)):
                t = wpool.tile([128, KT * DC], BF, name=f"w{wname}")
                w_tiles[wname] = (t, wd)
                for k in range(KT):
                    w_sb[(wname, k)] = t[:, k * DC:(k + 1) * DC]

            def load_w(wname):
                t, wd = w_tiles[wname]
                nc.sync.dma_start(t[:], wd[:])

            load_w("q")

            selb_sb = consts.tile([128, 128], BF, name="selb_sb")
            selbq_sb = consts.tile([128, 128], BF, name="selbq_sb")
            ident_sb = consts.tile([128, 128], BF, name="ident_sb")
            negtri_sb = consts.tile([128, 128], BF, name="negtri_sb")

            def load_consts():
                nc.sync.dma_start(selb_sb[:], selb_d[:])
                nc.sync.dma_start(selbq_sb[:], selbq_d[:])
                nc.sync.dma_start(ident_sb[:], ident_d[:])
                nc.sync.dma_start(negtri_sb[:], negtri_d[:])

            eps_sb = consts.tile([128, 1], F32, name="eps_sb")
            nc.vector.memset(eps_sb[:], EPS)
            zero_sb = consts.tile([128, 1], F32, name="zero_sb")
            nc.vector.memset(zero_sb[:], 0.0)
            ebias_sb = consts.tile([128, 1], F32, name="ebias_sb")
            nc.vector.memset(ebias_sb[:], EXP_BIAS)
            # wo is only needed by the first output projection (~half-way in);
            # its 2MB DMA is deferred into the pipeline so it doesn't delay
            # the first x-chunk prefetches behind it in the queue.
            wo_sb = wpool.tile([128, KT * D], BF, name="wo_sb")
            wo_loaded = [False]

            def load_wo():
                if not wo_loaded[0]:
                    wo_loaded[0] = True
                    nc.sync.dma_start(wo_sb[:], wo_d[:])

            # DRAM staging for ONE AllToAll per BATCH (batch 0's exchange +
            # output projection then hide under batch 1's attention).
            # Row-block j = [my 128 features for row-range j of batch b];
            # after the AllToAll, block s = core s's features for THIS
            # core's row ranges of batch b.
            a_dr = [dram.tile([N_CORES * DC, RPB], BF, name=f"a_dr{b}")
                    for b in range(B)]
            if collective:
                g_dr = [dram.tile([N_CORES * DC, RPB], BF, name=f"g_dr{b}")
                        for b in range(B)]
            else:
                g_dr = a_dr  # collective-free variant for TimelineSim

            # per-chunk normalized q|k bf16, feature-major: one [128, 1024]
            # tile per chunk, q*rstd/sqrt(DH) at cols [0:512], k*rstd at
            # [512:1024] (QSCALE folds into the q-half sumsq selector).
            qkns = {}   # (b, i) -> [DC, 2*SCHUNK] bf16
            vaug = {}   # (b, j) -> [128, 65] bf16 slice: [v|1] per head
            pqks = {}   # (b, i) -> in-flight q|k projection PSUM pair

            xcs = {}
            rep_box = [0]

            def prefetch_x(b, i):
                rep = rep_box[0]
                col0 = b * S + i * SCHUNK
                xc = xcp.tile([128, KT * SCHUNK], BF, name=f"x_{rep}_{b}_{i}",
                              tag="xc")
                # one DMA per k-tile: early projection parts can start
                # before the rest of the chunk lands
                if "xdma" not in abl:
                    for k in range(KT):
                        nc.sync.dma_start(
                            xc[:, k * SCHUNK:(k + 1) * SCHUNK],
                            xt_d[k * 128:(k + 1) * 128, col0:col0 + SCHUNK])
                else:
                    nc.vector.memset(xc[:, 0:8], 0.0)
                xcs[(b, i)] = xc

            def proj_qk_mm(b, i, xch, half, ks):
                rep = rep_box[0]
                if half == 0 and ks[0] == 0:
                    pqks[(b, i)] = ps.tile(
                        [128, 2 * SCHUNK], F32, name=f"pqk_{rep}_{b}_{i}",
                        tag="pair", bufs=3)
                pqk = pqks[(b, i)]
                wname = "qk"[half]
                for k in ks:
                    nc.tensor.matmul(
                        pqk[:, half * SCHUNK:(half + 1) * SCHUNK],
                        w_sb[(wname, k)][:], xch[k][:], start=(k == 0),
                        stop=(k == KT - 1))

            # The normalize chain is split into THREE weave parts so its PE
            # matmuls (ssbc sumsq, rstd broadcast) are issued a few j-tiles
            # AFTER their DVE/ACT producers and never head-of-line block the
            # in-order PE queue: norm_a (DVE: qkr copy + square), norm_b1
            # (PE sumsq + ACT Ln/Exp), norm_b2 (PE broadcast + DVE muls).
            norm_st = {}

            def proj_qk_norm_a(b, i):
                rep = rep_box[0]
                # raw q|k off PSUM in ONE copy (frees the pair slot); the
                # normalize chain then runs in bf16.
                qkr = sqp.tile([128, 2 * SCHUNK], BF,
                               name=f"qkr_{rep}_{b}_{i}", tag="qkr")
                sq = sqp.tile([128, 2 * SCHUNK], BF,
                              name=f"sq_{rep}_{b}_{i}", tag="sq")
                norm_st[(b, i)] = (qkr, sq)
                pqk = pqks.pop((b, i))
                if "norm" in abl:
                    nc.vector.memset(qkr[:, 0:8], 0.0)
                    nc.vector.memset(sq[:, 0:8], 0.0)
                    return
                nc.vector.tensor_copy(qkr[:], pqk[:])
                # square on DVE (bf16 2x), not ACT: keeps the sumsq matmul's
                # dependency off the exp-saturated ACT queue
                nc.vector.tensor_mul(sq[:], qkr[:], qkr[:])

            def proj_qk_norm_b1(b, i):
                if "norm" in abl:
                    return
                rep = rep_box[0]
                qkr, sq = norm_st[(b, i)]
                # selb has 1/DH in each head's 64x64 diagonal block, so ONE
                # matmul per half yields the per-head MEAN of squares already
                # broadcast over the head's 64 rows (output rows cost the
                # same regardless of partition count). The q half uses selbq
                # (entries 1.0 = 1/DH * DH) so its rstd comes out scaled by
                # 1/sqrt(DH): QSCALE folds in for free and q and k normalize
                # in ONE multiply downstream.
                ssbc = ps.tile([128, 2 * SCHUNK], F32, name=f"ssbc_{rep}_{b}_{i}",
                               tag="pair", bufs=3)
                for half, sel in ((0, selbq_sb), (1, selb_sb)):
                    nc.tensor.matmul(
                        ssbc[:, half * SCHUNK:(half + 1) * SCHUNK], sel[:],
                        sq[:, half * SCHUNK:(half + 1) * SCHUNK],
                        start=True, stop=True)
                lm = stdp.tile([128, 2 * SCHUNK], F32, name=f"lm_{rep}_{b}_{i}",
                               tag="lm")
                nc.scalar.activation(lm[:], ssbc[:],
                                     mybir.ActivationFunctionType.Ln,
                                     bias=eps_sb[:])
                rstd = bcp.tile([128, 2 * SCHUNK], BF, name=f"rstd_{rep}_{b}_{i}",
                                tag="rstd")
                nc.scalar.activation(rstd[:], lm[:],
                                     mybir.ActivationFunctionType.Exp,
                                     scale=-0.5, bias=zero_sb[:])
                norm_st[(b, i)] = (qkr, rstd)

            def proj_qk_norm_b2(b, i):
                rep = rep_box[0]
                qkn = persist.tile([DC, 2 * SCHUNK], BF, name=f"qkn_{rep}_{b}_{i}",
                                   tag="qkn", bufs=8)
                qkns[(b, i)] = qkn
                if "norm" in abl:
                    nc.vector.memset(qkn[:, 0:8], 0.0)
                    return
                qkr, rstd = norm_st.pop((b, i))
                nc.vector.tensor_mul(qkn[:], qkr[:], rstd[:])

            def proj_v_mm(b, i, xch, ks):
                rep = rep_box[0]
                if ks[0] == 0:
                    pqks[(b, i, "v")] = ps.tile(
                        [128, 2 * SCHUNK], F32, name=f"pv_{rep}_{b}_{i}",
                        tag="pair", bufs=3)
                psv = pqks[(b, i, "v")]
                for k in ks:
                    nc.tensor.matmul(psv[:, 0:SCHUNK], w_sb[("v", k)][:],
                                     xch[k][:], start=(k == 0),
                                     stop=(k == KT - 1))

            def proj_v_fin(b, i):
                rep = rep_box[0]
                psv = pqks.pop((b, i, "v"))
                vt = vtp.tile([DC, SCHUNK], BF, name=f"vt_{rep}_{b}_{i}",
                              tag="vt")
                nc.vector.tensor_copy(vt[:], psv[:, 0:SCHUNK])
                # transposes reuse the (dead) second bank of the psv slot —
                # their outputs land CONTIGUOUSLY so one DVE copy + one
                # memset assembles all four [v|1] j-tiles of the chunk.
                # (A DMA-xbar transpose was tried instead: ~1.9us init per
                # transfer in the backend's cost model made it far worse.)
                for u in range(SCHUNK // TT):
                    tpb = psv[:, SCHUNK + 64 * u:SCHUNK + 64 * (u + 1)].bitcast(BF)
                    nc.tensor.transpose(tpb[:], vt[:, u * 128:(u + 1) * 128],
                                        ident_sb[:])
                va4 = vaugp.tile([128, 4 * 2 * (DH + 1)], BF,
                                 name=f"va_{rep}_{b}_{i}", tag="vaug")
                va4v = va4[:].rearrange("p (u g d) -> p u g d", u=4, g=2)
                nc.vector.tensor_copy(
                    va4v[:, :, :, 0:DH],
                    psv[:, SCHUNK:SCHUNK + 4 * 64].bitcast(BF)[:]
                    .rearrange("p (u g d) -> p u g d", u=4, g=2))
                nc.vector.memset(va4v[:, :, :, DH:DH + 1], 1.0)
                for u in range(SCHUNK // TT):
                    j = i * (SCHUNK // TT) + u
                    vaug[(b, j)] = va4[:, u * 2 * (DH + 1):(u + 1) * 2 * (DH + 1)]

            def proj_parts(b, i):
                xc = xcs.pop((b, i))
                xch = [xc[:, k * SCHUNK:(k + 1) * SCHUNK] for k in range(KT)]
                # (pe_cost, fn) — cost in 512-col matmul units, used to pace
                # the weave by PE work rather than by part count
                return [
                    (2, lambda: proj_qk_mm(b, i, xch, 0, [0, 1])),
                    (2, lambda: proj_qk_mm(b, i, xch, 0, [2, 3])),
                    (2, lambda: proj_qk_mm(b, i, xch, 0, [4, 5])),
                    (2, lambda: proj_qk_mm(b, i, xch, 0, [6, 7])),
                    (2, lambda: proj_qk_mm(b, i, xch, 1, [0, 1])),
                    (2, lambda: proj_qk_mm(b, i, xch, 1, [2, 3])),
                    (2, lambda: proj_qk_mm(b, i, xch, 1, [4, 5])),
                    (2, lambda: proj_qk_mm(b, i, xch, 1, [6, 7])),
                    (0, lambda: proj_qk_norm_a(b, i)),
                    (2, lambda: proj_v_mm(b, i, xch, [0, 1])),
                    (2, lambda: proj_v_mm(b, i, xch, [2, 3])),
                    (2, lambda: proj_qk_norm_b1(b, i)),
                    (2, lambda: proj_v_mm(b, i, xch, [4, 5])),
                    (2, lambda: proj_v_mm(b, i, xch, [6, 7])),
                    (0, lambda: proj_qk_norm_b2(b, i)),
                    (1, lambda: proj_v_fin(b, i)),
                ]

            def do_proj(b, i):
                for _, part in proj_parts(b, i):
                    part()

            def do_attn(b, i, weave=None):
                rep = rep_box[0]
                # ONE [65, 1024] accumulator spanning 2 banks, head h at
                # cols [512h, 512h+512): halves the attnorm op count
                att = ps.tile([DH + 1, 2 * SCHUNK], F32,
                              name=f"att_{rep}_{b}_{i}", tag="att", bufs=1)
                n_t = 4 * i + 4
                parts = list(weave) if weave else []
                wtot = sum(c for c, _ in parts) or 1
                wdone = [0.0]
                psbs = {}

                def pv(j):
                    # PV for tile j, one software-pipeline stage behind the
                    # exp so the PE never waits on the current tile's exp
                    offj = max(0, TT * (j - 4 * i))
                    pj = psbs.pop(j)
                    for h in range(HEADS_PER_CORE):
                        nc.tensor.matmul(
                            att[:, SCHUNK * h + offj:SCHUNK * (h + 1)],
                            vaug[(b, j)][:, h * (DH + 1):(h + 1) * (DH + 1)],
                            pj[:, SCHUNK * h + offj:SCHUNK * (h + 1)],
                            start=(j == 0), stop=(j == n_t - 1),
                        )

                pts = {}

                def qk(j):
                    off = max(0, TT * (j - 4 * i))
                    jc, ju = j // 4, j % 4
                    diag = j >= 4 * i
                    # both heads' scores in one 2-bank pair tile: head h at
                    # cols [512h+off, 512h+512)
                    pt = ps.tile([128, 2 * SCHUNK], F32,
                                 name=f"ptile_{rep}_{b}_{i}_{j}", tag="pair", bufs=3)
                    pts[j] = pt
                    qkn_i = qkns[(b, i)]
                    qkn_j = qkns[(b, jc)]
                    for h in range(HEADS_PER_CORE):
                        nc.tensor.matmul(
                            pt[:, SCHUNK * h + off:SCHUNK * (h + 1)],
                            qkn_j[h * DH:(h + 1) * DH,
                                  SCHUNK + ju * TT:SCHUNK + (ju + 1) * TT],
                            qkn_i[h * DH:(h + 1) * DH, off:SCHUNK],
                            start=True, stop=not diag,
                            tile_position=(h * DH, 0),
                        )
                        if diag:
                            # causal mask as a PE accumulate: -1000 on the
                            # strictly-upper triangle of the diagonal block
                            # (exp underflows to exactly 0); frees the DVE
                            # of per-tile mask multiplies
                            nc.tensor.matmul(
                                pt[:, SCHUNK * h + off:SCHUNK * h + off + TT],
                                negtri_sb[:], ident_sb[:],
                                start=False, stop=True)

                def expmask(j):
                    off = max(0, TT * (j - 4 * i))
                    pt = pts.pop(j)
                    # one exp covers both heads via the [128, 2, npx] view
                    psb = pp.tile([128, 2 * SCHUNK], BF,
                                  name=f"p_{rep}_{b}_{i}_{j}", tag="p")
                    psbs[j] = psb
                    if "exp" in abl:
                        nc.vector.memset(psb[:, 0:8], 0.0)
                        return
                    ptv = pt[:].rearrange("p (h c) -> p h c", h=2)
                    psv = psb[:].rearrange("p (h c) -> p h c", h=2)
                    nc.scalar.activation(
                        psv[:, :, off:SCHUNK], ptv[:, :, off:SCHUNK],
                        mybir.ActivationFunctionType.Exp, bias=ebias_sb[:])

                # deepened inner pipeline: QK(j) runs one tile ahead of
                # exp(j-1) and two ahead of PV(j-2), so the PE never sits on
                # the first exp at a chunk start while the pipeline refills
                for j in range(n_t):
                    qk(j)
                    if j >= 1:
                        expmask(j - 1)
                    # weave projection/output-projection work here, paced by
                    # cumulative PE cost across j-tiles so the PE stays fed
                    # while ACT runs the exp chain
                    want = wtot * (j + 1) / n_t
                    while parts and wdone[0] < want:
                        c, fn = parts.pop(0)
                        wdone[0] += max(c, 0.25)
                        fn()
                    if j >= 3:
                        pv(j - 3)
                expmask(n_t - 1)
                for j in range(max(0, n_t - 3), n_t):
                    pv(j)

                for _, part in parts:
                    part()

                # ---- normalize by softmax denominator ----
                # at_all64 is [64 v-dims, (h, q)]-major; the a_dr store DMA
                # scatters (h, d) back into the 128-feature row blocks.
                at_all = attallp.tile([DH, 2 * SCHUNK], BF,
                                      name=f"atall_{rep}_{b}_{i}", tag="attall")
                if "attnorm" in abl:
                    nc.vector.memset(at_all[:, 0:8], 0.0)
                elif i == NCH - 1:
                    # last chunk of the batch: this chain gates the AllToAll
                    # (nothing overlaps it), so take the low-latency path —
                    # read the PSUM accumulator directly on DVE and skip the
                    # asb staging copy.
                    rc = rcp.tile([1, 2 * SCHUNK], F32,
                                  name=f"rc_{rep}_{b}_{i}", tag="rc")
                    nc.vector.reciprocal(rc[:], att[DH:DH + 1, :])
                    zbs = zbp.tile([DH, 2 * SCHUNK], F32,
                                   name=f"zbs_{rep}_{b}_{i}", tag="zb")
                    nc.gpsimd.partition_broadcast(zbs[:], rc[:])
                    nc.vector.tensor_mul(at_all[:], att[0:DH, :], zbs[:])
                else:
                    # copy the accumulator off PSUM immediately (releases
                    # the att banks for the next chunk without waiting on
                    # the normalize chain); normalize runs on Pool/GPSIMD
                    # to decongest DVE mid-pipeline.
                    asb = attsbp.tile([DH + 1, 2 * SCHUNK], F32,
                                      name=f"asb_{rep}_{b}_{i}", tag="asb")
                    nc.vector.tensor_copy(asb[:], att[:])
                    rc = rcp.tile([1, 2 * SCHUNK], F32,
                                  name=f"rc_{rep}_{b}_{i}", tag="rc")
                    nc.vector.reciprocal(rc[:], asb[DH:DH + 1, :])
                    zbs = zbp.tile([DH, 2 * SCHUNK], F32,
                                   name=f"zbs_{rep}_{b}_{i}", tag="zb")
                    nc.gpsimd.partition_broadcast(zbs[:], rc[:])
                    nc.gpsimd.tensor_mul(at_all[:], asb[0:DH, :], zbs[:])

                # stage this chunk's attention output for its batch's
                # AllToAll: chunk i covers row-ranges 2i and 2i+1;
                # a_dr row (2i+half)*128 + 64h + d <- at_all64[d, 512h +
                # 256*half + c] (one store per half keeps the APs 3-D)
                av = at_all[:].rearrange("d (h half c) -> d h half c",
                                         h=2, half=2)
                for half in range(2):
                    r0 = (2 * i + half) * DC
                    nc.sync.dma_start(
                        a_dr[b][r0:r0 + DC, :]
                        .rearrange("(h d) c -> d h c", h=2),
                        av[:, :, half, :])

            def do_a2a(b):
                if collective:
                    nc.gpsimd.collective_compute(
                        "AllToAll",
                        mybir.AluOpType.bypass,
                        replica_groups=[list(range(N_CORES))],
                        ins=[a_dr[b][:]],
                        outs=[g_dr[b][:]],
                    )

            def outproj_parts(b):
                rep = rep_box[0]
                # load gathered A^T [1024 feats, 256 rows] as k-tile blocks
                gsb = gsp.tile([128, KT * RPB], BF, name=f"gsb_{rep}_{b}",
                               tag="gsb")
                # two half-loads so the first matmuls (k-tiles 0-3) start a
                # DMA earlier at the tail
                hk = KT // 2
                for u in range(2):
                    nc.sync.dma_start(
                        gsb[:, u * hk * RPB:(u + 1) * hk * RPB]
                        .rearrange("p (k c) -> p k c", k=hk),
                        g_dr[b][u * hk * 128:(u + 1) * hk * 128, :]
                        .rearrange("(k p) c -> p k c", p=128))
                ops = {}

                def mm(rt, n, ks):
                    if n == 0 and ks[0] == 0:
                        ops[rt] = ps.tile([128, 2 * SCHUNK], F32,
                                          name=f"op_{rep}_{b}_{rt}",
                                          tag="pair", bufs=3)
                    if "opmm" in abl:
                        if n == 0 and ks[0] == 0:
                            nc.vector.memset(ops[rt][:, 0:8], 0.0)
                        return
                    for k in ks:
                        nc.tensor.matmul(
                            ops[rt][:, n * SCHUNK:(n + 1) * SCHUNK],
                            gsb[:, RPB * k + TT * rt:RPB * k + TT * (rt + 1)],
                            wo_sb[:, D * k + SCHUNK * n:
                                  D * k + SCHUNK * (n + 1)],
                            start=(k == 0), stop=(k == KT - 1))

                def fin(rt):
                    # (PSUM->DRAM DMA is not supported; stage through SBUF)
                    osb = outsbp.tile([128, D], F32, name=f"osb_{rep}_{b}_{rt}",
                                      tag="osb")
                    nc.vector.tensor_copy(osb[:], ops.pop(rt)[:])
                    nc.sync.dma_start(
                        out_d[(2 * b + rt) * TT:(2 * b + rt + 1) * TT, :],
                        osb[:])

                parts = []
                for rt in range(2):
                    for n in range(2):
                        parts.append((4, lambda rt=rt, n=n: mm(rt, n, [0, 1, 2, 3])))
                        parts.append((4, lambda rt=rt, n=n: mm(rt, n, [4, 5, 6, 7])))
                    parts.append((0, lambda rt=rt: fin(rt)))
                return parts

            # Software pipeline, one FLAT stream of steps across all repeats
            # (no rep-boundary special cases). Step s: attention for global
            # chunk s-1, WOVEN with the projection parts for chunk s+1 (two
            # steps of slack for the normalize chain's DVE/ACT latency) and
            # any output-projection parts scheduled for this step; x for
            # chunk s+2 is prefetched. A batch's AllToAll is issued as soon
            # as its last chunk's attention has staged its output, and its
            # output projection weaves in two steps later so the in-order PE
            # queue never head-of-line blocks on the exchange.
            chunks = [(b, i) for b in range(B) for i in range(NCH)]
            nch = len(chunks)
            total = repeat * nch
            pending = {}
            pf = [0]

            def pf_to(limit):
                while pf[0] < min(limit, total):
                    rep_box[0] = pf[0] // nch
                    prefetch_x(*chunks[pf[0] % nch])
                    pf[0] += 1

            # preamble DMA order: wq (issued at tile creation above) ->
            # x chunk 0 -> wk, wv -> consts; the first projection can then
            # start as soon as wq + the first x k-tiles land (~2us).
            pf_to(1)
            load_w("k")
            load_w("v")
            load_consts()

            for s in range(total + 1):
                pf_to(s + 3)
                if s == 1:
                    load_wo()
                weave = []
                if s + 1 < total:
                    rep_box[0] = (s + 1) // nch
                    weave += proj_parts(*chunks[(s + 1) % nch])
                weave += pending.pop(s, [])
                if s >= 1 and stage >= 2:
                    g = s - 1
                    rep_box[0] = g // nch
                    b_prev, i_prev = chunks[g % nch]
                    do_attn(b_prev, i_prev, weave=weave)
                    if stage >= 3 and i_prev == NCH - 1:
                        # this batch's last chunk is staged: exchange now;
                        # its output projection weaves in two steps later
                        # (batch 0's entirely overlaps batch 1's attention)
                        do_a2a(b_prev)
                        pending[s + 2] = outproj_parts(b_prev)
                else:
                    rep_box[0] = 0
                    do_proj(*chunks[0])
                    for _, part in weave:
                        part()
            for parts in pending.values():
                for _, part in parts:
                    part()

    nc.compile()
    return nc


_NC_CACHE = {}


def _get_nc():
    if "nc" not in _NC_CACHE:
        _NC_CACHE["nc"] = build_nc()
    return _NC_CACHE["nc"]


def _ktile_layout(w):
    # [K*128, C] -> [128, K*C]: free block k holds rows [128k, 128k+128)
    kt, c = w.shape[0] // 128, w.shape[1]
    return np.ascontiguousarray(
        w.reshape(kt, 128, c).transpose(1, 0, 2).reshape(128, kt * c))


def _host_inputs(x, Wq, Wk, Wv, Wo):
    xt = np.ascontiguousarray(
        np.asarray(x, dtype=np.float32).reshape(ROWS, D).T).astype(BF16)
    ident = np.eye(128, dtype=BF16)
    # causal-mask accumulate: out[m,n] += negtri[n,m] = -1000 iff key m >
    # query n (strictly upper triangle); exp then underflows to exactly 0
    negtri = np.triu(np.full((128, 128), -1000.0, dtype=np.float32), 1).astype(BF16)
    # block-diagonal mean-of-squares selector: selb[d, m] = 1/DH when d and
    # m fall in the same 64-row head group (output = per-head mean already
    # broadcast over the head's rows); selbq additionally folds the
    # 1/sqrt(DH) score scale into the q-half rstd (entries 1.0 = DH/DH)
    selb = np.zeros((128, 128), dtype=np.float32)
    for h in range(HEADS_PER_CORE):
        selb[h * DH:(h + 1) * DH, h * DH:(h + 1) * DH] = 1.0 / DH
    selbq = (selb * DH).astype(BF16)
    selb = selb.astype(BF16)
    wo_h = _ktile_layout(np.asarray(Wo, dtype=np.float32)).astype(BF16)

    in_maps = []
    for c in range(N_CORES):
        cs = c * DC
        in_maps.append({
            "xt": xt,
            "wq": _ktile_layout(np.asarray(Wq, dtype=np.float32)[:, cs:cs + DC]).astype(BF16),
            "wk": _ktile_layout(np.asarray(Wk, dtype=np.float32)[:, cs:cs + DC]).astype(BF16),
            "wv": _ktile_layout(np.asarray(Wv, dtype=np.float32)[:, cs:cs + DC]).astype(BF16),
            "wo": wo_h,
            "negtri": negtri,
            "ident": ident,
            "selb": selb,
            "selbq": selbq,
        })
    return in_maps


def kernel(x, Wq, Wk, Wv, Wo, mask):
    x = np.asarray(x, dtype=np.float32)
    nc = _get_nc()
    in_maps = _host_inputs(x, np.asarray(Wq), np.asarray(Wk),
                           np.asarray(Wv), np.asarray(Wo))
    res = run_bass_kernel_spmd(nc, in_maps, list(range(N_CORES)))
    full = np.empty((ROWS, D), dtype=np.float32)
    for c in range(N_CORES):
        o = res.results[c]["out"]
        for b in range(B):
            r0 = b * S + c * RPB
            full[r0:r0 + RPB] = o[b * RPB:(b + 1) * RPB]
    return full.reshape(B, S, D)


if __name__ == "__main__":
    nc = build_nc()
    print("kernel built and compiled OK")



# revision 71
# speedup vs baseline: 1.1856x; 1.0283x over previous
"""Trainium2 Bass kernel for nn_Attention_47545287967487.

Causal multi-head attention (B=2, S=2048, D=1024, H=16, DH=64) with QK
RMS-norm, distributed over 8 NeuronCores.

Distribution: head tensor-parallel for the QKV projections and attention
(each core owns 2 heads = a 128-column slice of Wq/Wk/Wv, computing the
full 4096-row sequence), then ONE AllToAll per iteration redistributes
the bf16 attention outputs of both batches so each core owns a 256-row
slice per batch and runs the output projection locally with the full
Wo. This moves ~1MB/core over the fabric instead of ReduceScattering a
16MB fp32 partial, the final out write is 2MB instead of 16MB, and a
single merged exchange pays the collective launch cost (~5-6us
measured on HW) once instead of twice.

Numerics: x/W/QK^T/PV run in bf16 (fp32 PSUM accumulation), softmax in
fp32->bf16. Scores are bounded (|q.k|/8 <= 8 after RMS-norm) so softmax
skips the max-subtraction pass; a constant -2.25 bias inside the exp is
harmless (it cancels in the normalization). An fp8e4m3 P/V variant with
DoubleRow PV was tried and rejected: it measured rel err 2.7e-2 against
the 2e-2 gate (P/V quantization ~4% each).

Engine plan per core:
 - PE: x@W projections (bf16, K-tiled), QK^T with the two heads packed
   into array row-groups (tile_position), PV as [v|1]^T @ P so the
   softmax denominator is a free 65th output row, the local output
   projection, selector matmuls for the q+k sum-of-squares rows and the
   rstd broadcast, and 128x128 transposes of v into [t,d] layout.
 - ACT: one exp per (chunk, key-tile) covering BOTH heads via a
   [128,2,npx] access pattern over a 2-bank PSUM pair tile; ONE
   Square / Ln / Exp per chunk for the q+k rstd chain (q and k side by
   side on partitions 0:2 of a [2,1024] tile). The 1/sqrt(DH) score
   scale folds into the q normalize multiply.
 - DVE: PSUM->SBUF casts/copies on the critical projection chain, q/k
   normalize multiplies, causal mask multiplies (only the 128-col
   diagonal block), and softmax denominator reciprocals.
 - Pool/GPSIMD: the AllToAll collective, denominator partition
   broadcasts, and the attention-output normalize multiplies (these
   read only SBUF — GPSIMD cannot touch PSUM — and have a full
   iteration of slack before their a_dr-store consumers, so they
   tolerate Pool stalls and decongest DVE, whose queue otherwise
   delays the projection normalize chain).

Scheduling: one FLAT stream of steps across all repeats (so repeat
tails overlap the next repeat's head) that software-pipelines at three
levels. Chunk c's projection parts (~8) are WOVEN between the j-tiles
of chunk c-2's attention, paced evenly, so the PE consumes them while
ACT runs the serial exp chain and the normalize chain's DVE/ACT
latency is hidden by a full extra step; within a chunk the QK matmul
runs two j-tiles AHEAD of the exp (the 3-slot score-pair budget's max)
and the PV matmul one behind it, so the PE never sits on an exp even
while the pipeline refills at chunk starts; and the output-projection parts weave in two and three steps
after the AllToAll so the in-order PE queue never head-of-line blocks
on the exchange. The raw q|k projection is copied off PSUM immediately
(qkr), releasing its pair slot in ~1.2us and letting the whole
normalize chain run in bf16.

PSUM discipline: exactly 8 banks = one pool with a [128,1024] fp32
"pair" tag (bufs=3; holds q+k projection pairs, v projections + v
transposes, the sumsq/broadcast scratch, score pairs, and output-
projection pairs) plus two [65,512] attention accumulators.

kernel(**inputs) takes the FULL unsharded inputs and returns the FULL
[2, 2048, 1024] float32 output.
"""

import numpy as np

import concourse.bacc as bacc
import concourse.mybir as mybir
from concourse import tile
from concourse.bass_utils import run_bass_kernel_spmd

import ml_dtypes

BF16 = ml_dtypes.bfloat16

# Problem shape (hardcoded per the harness contract).
B, S, D, DH = 2, 2048, 1024, 64
H = D // DH
N_CORES = 8
HEADS_PER_CORE = H // N_CORES          # 2
DC = HEADS_PER_CORE * DH               # 128 feature columns per core
EPS = 1e-6

SCHUNK = 512                            # s-chunk width
TT = 128                                # t-tile width
KT = D // 128                           # 8 contraction tiles
NCH = S // SCHUNK                       # 4 s-chunks per batch
ROWS = B * S                            # 4096
RPB = S // N_CORES                      # 256 output rows per core per batch
EXP_BIAS = -2.25                        # softmax headroom shift (cancels)
QSCALE = 1.0 / (DH ** 0.5)              # folded into the q normalize

F32 = mybir.dt.float32
F32R = mybir.dt.float32r
BF = mybir.dt.bfloat16

# All ACT functions this kernel uses (Square, Ln, Exp, Copy) live in the
# 'natural_log_exp_and_others' table. The default table chooser picks the
# first table containing each function, which thrashes between the exp and
# ln tables (~1.3us per reload, dozens of reloads). Pin the chooser to the
# one table that covers everything by emptying the others (positions are
# preserved so act_func_set_id still indexes act_info.json correctly).
_PINNED_ACT_TABLE = "natural_log_exp_and_others"
_orig_get_act_tables = bacc.get_activation_tables


def _pinned_act_tables(arch):
    tables = _orig_get_act_tables(arch)
    return {
        name: (funcs if name == _PINNED_ACT_TABLE else set())
        for name, funcs in tables.items()
    }


bacc.get_activation_tables = _pinned_act_tables


def build_nc(collective=True, stage=3, repeat=1, abl=()):
    # abl: timing-only ablation flags ("xdma", "exp", "norm", "attnorm",
    # "opmm") — skip instruction classes to attribute backend time. Output
    # is garbage when any flag is set; used by ablate.py only.
    abl = frozenset(abl)
    nc = bacc.Bacc("TRN2", target_bir_lowering=False)

    xt_d = nc.dram_tensor("xt", [D, ROWS], BF, kind="ExternalInput")
    # weights are host-transposed to the SBUF layout [128, KT*cols]: free
    # block k holds W rows [128k, 128k+128), so the DMA is a straight copy
    # with 2KB+ contiguous lines (256B lines pay a 2x DMA latency penalty).
    wq_d = nc.dram_tensor("wq", [128, KT * DC], BF, kind="ExternalInput")
    wk_d = nc.dram_tensor("wk", [128, KT * DC], BF, kind="ExternalInput")
    wv_d = nc.dram_tensor("wv", [128, KT * DC], BF, kind="ExternalInput")
    wo_d = nc.dram_tensor("wo", [128, KT * D], BF, kind="ExternalInput")
    negtri_d = nc.dram_tensor("negtri", [128, 128], BF, kind="ExternalInput")
    ident_d = nc.dram_tensor("ident", [128, 128], BF, kind="ExternalInput")
    selb_d = nc.dram_tensor("selb", [128, 128], BF, kind="ExternalInput")
    selbq_d = nc.dram_tensor("selbq", [128, 128], BF, kind="ExternalInput")
    out_d = nc.dram_tensor("out", [2 * RPB, D], F32, kind="ExternalOutput")

    from contextlib import ExitStack
    with tile.TileContext(nc) as tc:
        with ExitStack() as ctx:
            consts = ctx.enter_context(tc.tile_pool(name="consts", bufs=1))
            wpool = ctx.enter_context(tc.tile_pool(name="wpool", bufs=1))
            persist = ctx.enter_context(tc.tile_pool(name="persist", bufs=1))
            xcp = ctx.enter_context(tc.tile_pool(name="xc", bufs=3))
            sqp = ctx.enter_context(tc.tile_pool(name="sqp", bufs=2))
            stdp = ctx.enter_context(tc.tile_pool(name="stdp", bufs=2))
            bcp = ctx.enter_context(tc.tile_pool(name="bcp", bufs=2))
            vtp = ctx.enter_context(tc.tile_pool(name="vtp", bufs=2))
            vaugp = ctx.enter_context(tc.tile_pool(name="vaugp", bufs=10))
            pp = ctx.enter_context(tc.tile_pool(name="pp", bufs=8))
            attsbp = ctx.enter_context(tc.tile_pool(name="attsb", bufs=2))
            zbp = ctx.enter_context(tc.tile_pool(name="zbp", bufs=4))
            rcp = ctx.enter_context(tc.tile_pool(name="rcp", bufs=4))
            attallp = ctx.enter_context(tc.tile_pool(name="attall", bufs=3))
            gsp = ctx.enter_context(tc.tile_pool(name="gsp", bufs=2))
            outsbp = ctx.enter_context(tc.tile_pool(name="outsb", bufs=2))
            ps = ctx.enter_context(tc.tile_pool(name="ps", bufs=1, space="PSUM"))
            dram = ctx.enter_context(tc.tile_pool(name="dram", bufs=1, space="DRAM"))

            # ---- DMA issue order gates the pipeline head: wq first, then
            # the first x chunk (so the first projection parts can start
            # ~2us in), then wk/wv/consts. All transfers serialize on the
            # shared DMA-engine pool, so issue order == arrival order. ----
            w_sb = {}
            w_tiles = {}
            for wname, wd in (("q", wq_d), ("k", wk_d), ("v", wv_d)):
                t = wpool.tile([128, KT * DC], BF, name=f"w{wname}")
                w_tiles[wname] = (t, wd)
                for k in range(KT):
                    w_sb[(wname, k)] = t[:, k * DC:(k + 1) * DC]

            def load_w(wname):
                t, wd = w_tiles[wname]
                nc.sync.dma_start(t[:], wd[:])

            load_w("q")

            selb_sb = consts.tile([128, 128], BF, name="selb_sb")
            selbq_sb = consts.tile([128, 128], BF, name="selbq_sb")
            ident_sb = consts.tile([128, 128], BF, name="ident_sb")
            negtri_sb = consts.tile([128, 128], BF, name="negtri_sb")

            def load_consts():
                nc.sync.dma_start(selb_sb[:], selb_d[:])
                nc.sync.dma_start(selbq_sb[:], selbq_d[:])
                nc.sync.dma_start(ident_sb[:], ident_d[:])
                nc.sync.dma_start(negtri_sb[:], negtri_d[:])

            eps_sb = consts.tile([128, 1], F32, name="eps_sb")
            nc.vector.memset(eps_sb[:], EPS)
            zero_sb = consts.tile([128, 1], F32, name="zero_sb")
            nc.vector.memset(zero_sb[:], 0.0)
            ebias_sb = consts.tile([128, 1], F32, name="ebias_sb")
            nc.vector.memset(ebias_sb[:], EXP_BIAS)
            # wo is only needed by the first output projection (~half-way in);
            # its 2MB DMA is deferred into the pipeline so it doesn't delay
            # the first x-chunk prefetches behind it in the queue.
            wo_sb = wpool.tile([128, KT * D], BF, name="wo_sb")
            wo_loaded = [False]

            def load_wo():
                if not wo_loaded[0]:
                    wo_loaded[0] = True
                    nc.sync.dma_start(wo_sb[:], wo_d[:])

            # DRAM staging for ONE AllToAll per BATCH (batch 0's exchange +
            # output projection then hide under batch 1's attention).
            # Row-block j = [my 128 features for row-range j of batch b];
            # after the AllToAll, block s = core s's features for THIS
            # core's row ranges of batch b.
            a_dr = [dram.tile([N_CORES * DC, RPB], BF, name=f"a_dr{b}")
                    for b in range(B)]
            if collective:
                g_dr = [dram.tile([N_CORES * DC, RPB], BF, name=f"g_dr{b}")
                        for b in range(B)]
            else:
                g_dr = a_dr  # collective-free variant for TimelineSim

            # per-chunk normalized q|k bf16, feature-major: one [128, 1024]
            # tile per chunk, q*rstd/sqrt(DH) at cols [0:512], k*rstd at
            # [512:1024] (QSCALE folds into the q-half sumsq selector).
            qkns = {}   # (b, i) -> [DC, 2*SCHUNK] bf16
            vaug = {}   # (b, j) -> [128, 65] bf16 slice: [v|1] per head
            pqks = {}   # (b, i) -> in-flight q|k projection PSUM pair

            xcs = {}
            rep_box = [0]

            def prefetch_x(b, i):
                rep = rep_box[0]
                col0 = b * S + i * SCHUNK
                xc = xcp.tile([128, KT * SCHUNK], BF, name=f"x_{rep}_{b}_{i}",
                              tag="xc")
                # one DMA per k-tile: early projection parts can start
                # before the rest of the chunk lands
                if "xdma" not in abl:
                    for k in range(KT):
                        nc.sync.dma_start(
                            xc[:, k * SCHUNK:(k + 1) * SCHUNK],
                            xt_d[k * 128:(k + 1) * 128, col0:col0 + SCHUNK])
                else:
                    nc.vector.memset(xc[:, 0:8], 0.0)
                xcs[(b, i)] = xc

            def proj_qk_mm(b, i, xch, half, ks):
                rep = rep_box[0]
                if half == 0 and ks[0] == 0:
                    pqks[(b, i)] = ps.tile(
                        [128, 2 * SCHUNK], F32, name=f"pqk_{rep}_{b}_{i}",
                        tag="pair", bufs=3)
                pqk = pqks[(b, i)]
                wname = "qk"[half]
                for k in ks:
                    nc.tensor.matmul(
                        pqk[:, half * SCHUNK:(half + 1) * SCHUNK],
                        w_sb[(wname, k)][:], xch[k][:], start=(k == 0),
                        stop=(k == KT - 1))

            # The normalize chain is split into THREE weave parts so its PE
            # matmuls (ssbc sumsq, rstd broadcast) are issued a few j-tiles
            # AFTER their DVE/ACT producers and never head-of-line block the
            # in-order PE queue: norm_a (DVE: qkr copy + square), norm_b1
            # (PE sumsq + ACT Ln/Exp), norm_b2 (PE broadcast + DVE muls).
            norm_st = {}

            def proj_qk_norm_a(b, i):
                rep = rep_box[0]
                # raw q|k off PSUM in ONE copy (frees the pair slot); the
                # normalize chain then runs in bf16.
                qkr = sqp.tile([128, 2 * SCHUNK], BF,
                               name=f"qkr_{rep}_{b}_{i}", tag="qkr")
                sq = sqp.tile([128, 2 * SCHUNK], BF,
                              name=f"sq_{rep}_{b}_{i}", tag="sq")
                norm_st[(b, i)] = (qkr, sq)
                pqk = pqks.pop((b, i))
                if "norm" in abl:
                    nc.vector.memset(qkr[:, 0:8], 0.0)
                    nc.vector.memset(sq[:, 0:8], 0.0)
                    return
                nc.vector.tensor_copy(qkr[:], pqk[:])
                # square on DVE (bf16 2x), not ACT: keeps the sumsq matmul's
                # dependency off the exp-saturated ACT queue
                nc.vector.tensor_mul(sq[:], qkr[:], qkr[:])

            def proj_qk_norm_b1(b, i):
                if "norm" in abl:
                    return
                rep = rep_box[0]
                qkr, sq = norm_st[(b, i)]
                # selb has 1/DH in each head's 64x64 diagonal block, so ONE
                # matmul per half yields the per-head MEAN of squares already
                # broadcast over the head's 64 rows (output rows cost the
                # same regardless of partition count). The q half uses selbq
                # (entries 1.0 = 1/DH * DH) so its rstd comes out scaled by
                # 1/sqrt(DH): QSCALE folds in for free and q and k normalize
                # in ONE multiply downstream.
                ssbc = ps.tile([128, 2 * SCHUNK], F32, name=f"ssbc_{rep}_{b}_{i}",
                               tag="pair", bufs=3)
                for half, sel in ((0, selbq_sb), (1, selb_sb)):
                    nc.tensor.matmul(
                        ssbc[:, half * SCHUNK:(half + 1) * SCHUNK], sel[:],
                        sq[:, half * SCHUNK:(half + 1) * SCHUNK],
                        start=True, stop=True)
                lm = stdp.tile([128, 2 * SCHUNK], F32, name=f"lm_{rep}_{b}_{i}",
                               tag="lm")
                nc.scalar.activation(lm[:], ssbc[:],
                                     mybir.ActivationFunctionType.Ln,
                                     bias=eps_sb[:])
                rstd = bcp.tile([128, 2 * SCHUNK], BF, name=f"rstd_{rep}_{b}_{i}",
                                tag="rstd")
                nc.scalar.activation(rstd[:], lm[:],
                                     mybir.ActivationFunctionType.Exp,
                                     scale=-0.5, bias=zero_sb[:])
                norm_st[(b, i)] = (qkr, rstd)

            def proj_qk_norm_b2(b, i):
                rep = rep_box[0]
                qkn = persist.tile([DC, 2 * SCHUNK], BF, name=f"qkn_{rep}_{b}_{i}",
                                   tag="qkn", bufs=8)
                qkns[(b, i)] = qkn
                if "norm" in abl:
                    nc.vector.memset(qkn[:, 0:8], 0.0)
                    return
                qkr, rstd = norm_st.pop((b, i))
                nc.vector.tensor_mul(qkn[:], qkr[:], rstd[:])

            def proj_v_mm(b, i, xch, ks):
                rep = rep_box[0]
                if ks[0] == 0:
                    pqks[(b, i, "v")] = ps.tile(
                        [128, 2 * SCHUNK], F32, name=f"pv_{rep}_{b}_{i}",
                        tag="pair", bufs=3)
                psv = pqks[(b, i, "v")]
                for k in ks:
                    nc.tensor.matmul(psv[:, 0:SCHUNK], w_sb[("v", k)][:],
                                     xch[k][:], start=(k == 0),
                                     stop=(k == KT - 1))

            def proj_v_fin(b, i):
                rep = rep_box[0]
                psv = pqks.pop((b, i, "v"))
                vt = vtp.tile([DC, SCHUNK], BF, name=f"vt_{rep}_{b}_{i}",
                              tag="vt")
                nc.vector.tensor_copy(vt[:], psv[:, 0:SCHUNK])
                # transposes reuse the (dead) second bank of the psv slot —
                # their outputs land CONTIGUOUSLY so one DVE copy + one
                # memset assembles all four [v|1] j-tiles of the chunk.
                # (A DMA-xbar transpose was tried instead: ~1.9us init per
                # transfer in the backend's cost model made it far worse.)
                for u in range(SCHUNK // TT):
                    tpb = psv[:, SCHUNK + 64 * u:SCHUNK + 64 * (u + 1)].bitcast(BF)
                    nc.tensor.transpose(tpb[:], vt[:, u * 128:(u + 1) * 128],
                                        ident_sb[:])
                va4 = vaugp.tile([128, 4 * 2 * (DH + 1)], BF,
                                 name=f"va_{rep}_{b}_{i}", tag="vaug")
                va4v = va4[:].rearrange("p (u g d) -> p u g d", u=4, g=2)
                nc.vector.tensor_copy(
                    va4v[:, :, :, 0:DH],
                    psv[:, SCHUNK:SCHUNK + 4 * 64].bitcast(BF)[:]
                    .rearrange("p (u g d) -> p u g d", u=4, g=2))
                nc.vector.memset(va4v[:, :, :, DH:DH + 1], 1.0)
                for u in range(SCHUNK // TT):
                    j = i * (SCHUNK // TT) + u
                    vaug[(b, j)] = va4[:, u * 2 * (DH + 1):(u + 1) * 2 * (DH + 1)]

            def proj_parts(b, i):
                xc = xcs.pop((b, i))
                xch = [xc[:, k * SCHUNK:(k + 1) * SCHUNK] for k in range(KT)]
                # (pe_cost, fn) — cost in 512-col matmul units, used to pace
                # the weave by PE work rather than by part count
                return [
                    (2, lambda: proj_qk_mm(b, i, xch, 0, [0, 1])),
                    (2, lambda: proj_qk_mm(b, i, xch, 0, [2, 3])),
                    (2, lambda: proj_qk_mm(b, i, xch, 0, [4, 5])),
                    (2, lambda: proj_qk_mm(b, i, xch, 0, [6, 7])),
                    (2, lambda: proj_qk_mm(b, i, xch, 1, [0, 1])),
                    (2, lambda: proj_qk_mm(b, i, xch, 1, [2, 3])),
                    (2, lambda: proj_qk_mm(b, i, xch, 1, [4, 5])),
                    (2, lambda: proj_qk_mm(b, i, xch, 1, [6, 7])),
                    (0, lambda: proj_qk_norm_a(b, i)),
                    (2, lambda: proj_v_mm(b, i, xch, [0, 1])),
                    (2, lambda: proj_v_mm(b, i, xch, [2, 3])),
                    (2, lambda: proj_qk_norm_b1(b, i)),
                    (2, lambda: proj_v_mm(b, i, xch, [4, 5])),
                    (2, lambda: proj_v_mm(b, i, xch, [6, 7])),
                    (0, lambda: proj_qk_norm_b2(b, i)),
                    (1, lambda: proj_v_fin(b, i)),
                ]

            def do_proj(b, i):
                for _, part in proj_parts(b, i):
                    part()

            def do_attn(b, i, weave=None):
                rep = rep_box[0]
                # ONE [65, 1024] accumulator spanning 2 banks, head h at
                # cols [512h, 512h+512): halves the attnorm op count
                att = ps.tile([DH + 1, 2 * SCHUNK], F32,
                              name=f"att_{rep}_{b}_{i}", tag="att", bufs=1)
                n_t = 4 * i + 4
                parts = list(weave) if weave else []
                wtot = sum(c for c, _ in parts) or 1
                wdone = [0.0]
                psbs = {}

                def pv(j):
                    # PV for tile j, one software-pipeline stage behind the
                    # exp so the PE never waits on the current tile's exp
                    offj = max(0, TT * (j - 4 * i))
                    pj = psbs.pop(j)
                    for h in range(HEADS_PER_CORE):
                        nc.tensor.matmul(
                            att[:, SCHUNK * h + offj:SCHUNK * (h + 1)],
                            vaug[(b, j)][:, h * (DH + 1):(h + 1) * (DH + 1)],
                            pj[:, SCHUNK * h + offj:SCHUNK * (h + 1)],
                            start=(j == 0), stop=(j == n_t - 1),
                        )

                pts = {}

                def qk(j):
                    off = max(0, TT * (j - 4 * i))
                    jc, ju = j // 4, j % 4
                    diag = j >= 4 * i
                    # both heads' scores in one 2-bank pair tile: head h at
                    # cols [512h+off, 512h+512)
                    pt = ps.tile([128, 2 * SCHUNK], F32,
                                 name=f"ptile_{rep}_{b}_{i}_{j}", tag="pair", bufs=3)
                    pts[j] = pt
                    qkn_i = qkns[(b, i)]
                    qkn_j = qkns[(b, jc)]
                    for h in range(HEADS_PER_CORE):
                        nc.tensor.matmul(
                            pt[:, SCHUNK * h + off:SCHUNK * (h + 1)],
                            qkn_j[h * DH:(h + 1) * DH,
                                  SCHUNK + ju * TT:SCHUNK + (ju + 1) * TT],
                            qkn_i[h * DH:(h + 1) * DH, off:SCHUNK],
                            start=True, stop=not diag,
                            tile_position=(h * DH, 0),
                        )
                        if diag:
                            # causal mask as a PE accumulate: -1000 on the
                            # strictly-upper triangle of the diagonal block
                            # (exp underflows to exactly 0); frees the DVE
                            # of per-tile mask multiplies
                            nc.tensor.matmul(
                                pt[:, SCHUNK * h + off:SCHUNK * h + off + TT],
                                negtri_sb[:], ident_sb[:],
                                start=False, stop=True)

                def expmask(j):
                    off = max(0, TT * (j - 4 * i))
                    pt = pts.pop(j)
                    # one exp covers both heads via the [128, 2, npx] view
                    psb = pp.tile([128, 2 * SCHUNK], BF,
                                  name=f"p_{rep}_{b}_{i}_{j}", tag="p")
                    psbs[j] = psb
                    if "exp" in abl:
                        nc.vector.memset(psb[:, 0:8], 0.0)
                        return
                    ptv = pt[:].rearrange("p (h c) -> p h c", h=2)
                    psv = psb[:].rearrange("p (h c) -> p h c", h=2)
                    nc.scalar.activation(
                        psv[:, :, off:SCHUNK], ptv[:, :, off:SCHUNK],
                        mybir.ActivationFunctionType.Exp, bias=ebias_sb[:])

                # deepened inner pipeline: QK(j) runs one tile ahead of
                # exp(j-1) and two ahead of PV(j-2), so the PE never sits on
                # the first exp at a chunk start while the pipeline refills
                for j in range(n_t):
                    qk(j)
                    if j >= 1:
                        expmask(j - 1)
                    # weave projection/output-projection work here, paced by
                    # cumulative PE cost across j-tiles so the PE stays fed
                    # while ACT runs the exp chain
                    want = wtot * (j + 1) / n_t
                    while parts and wdone[0] < want:
                        c, fn = parts.pop(0)
                        wdone[0] += max(c, 0.25)
                        fn()
                    if j >= 3:
                        pv(j - 3)
                expmask(n_t - 1)
                for j in range(max(0, n_t - 3), n_t):
                    pv(j)

                for _, part in parts:
                    part()

                # ---- normalize by softmax denominator ----
                # at_all64 is [64 v-dims, (h, q)]-major; the a_dr store DMA
                # scatters (h, d) back into the 128-feature row blocks.
                at_all = attallp.tile([DH, 2 * SCHUNK], BF,
                                      name=f"atall_{rep}_{b}_{i}", tag="attall")
                if "attnorm" in abl:
                    nc.vector.memset(at_all[:, 0:8], 0.0)
                elif i == NCH - 1:
                    # last chunk of the batch: this chain gates the AllToAll
                    # (nothing overlaps it), so take the low-latency path —
                    # read the PSUM accumulator directly on DVE and skip the
                    # asb staging copy.
                    rc = rcp.tile([1, 2 * SCHUNK], F32,
                                  name=f"rc_{rep}_{b}_{i}", tag="rc")
                    nc.vector.reciprocal(rc[:], att[DH:DH + 1, :])
                    zbs = zbp.tile([DH, 2 * SCHUNK], F32,
                                   name=f"zbs_{rep}_{b}_{i}", tag="zb")
                    nc.gpsimd.partition_broadcast(zbs[:], rc[:])
                    nc.vector.tensor_mul(at_all[:], att[0:DH, :], zbs[:])
                else:
                    # copy the accumulator off PSUM immediately (releases
                    # the att banks for the next chunk without waiting on
                    # the normalize chain); normalize runs on Pool/GPSIMD
                    # to decongest DVE mid-pipeline.
                    asb = attsbp.tile([DH + 1, 2 * SCHUNK], F32,
                                      name=f"asb_{rep}_{b}_{i}", tag="asb")
                    nc.vector.tensor_copy(asb[:], att[:])
                    rc = rcp.tile([1, 2 * SCHUNK], F32,
                                  name=f"rc_{rep}_{b}_{i}", tag="rc")
                    nc.vector.reciprocal(rc[:], asb[DH:DH + 1, :])
                    zbs = zbp.tile([DH, 2 * SCHUNK], F32,
                                   name=f"zbs_{rep}_{b}_{i}", tag="zb")
                    nc.gpsimd.partition_broadcast(zbs[:], rc[:])
                    nc.gpsimd.tensor_mul(at_all[:], asb[0:DH, :], zbs[:])

                # stage this chunk's attention output for its batch's
                # AllToAll: chunk i covers row-ranges 2i and 2i+1;
                # a_dr row (2i+half)*128 + 64h + d <- at_all64[d, 512h +
                # 256*half + c] (one store per half keeps the APs 3-D)
                av = at_all[:].rearrange("d (h half c) -> d h half c",
                                         h=2, half=2)
                for half in range(2):
                    r0 = (2 * i + half) * DC
                    nc.sync.dma_start(
                        a_dr[b][r0:r0 + DC, :]
                        .rearrange("(h d) c -> d h c", h=2),
                        av[:, :, half, :])

            def do_a2a(b):
                if collective:
                    nc.gpsimd.collective_compute(
                        "AllToAll",
                        mybir.AluOpType.bypass,
                        replica_groups=[list(range(N_CORES))],
                        ins=[a_dr[b][:]],
                        outs=[g_dr[b][:]],
                    )

            def outproj_parts(b):
                rep = rep_box[0]
                # load gathered A^T [1024 feats, 256 rows] as k-tile blocks
                gsb = gsp.tile([128, KT * RPB], BF, name=f"gsb_{rep}_{b}",
                               tag="gsb")
                # two half-loads so the first matmuls (k-tiles 0-3) start a
                # DMA earlier at the tail
                hk = KT // 2
                for u in range(2):
                    nc.sync.dma_start(
                        gsb[:, u * hk * RPB:(u + 1) * hk * RPB]
                        .rearrange("p (k c) -> p k c", k=hk),
                        g_dr[b][u * hk * 128:(u + 1) * hk * 128, :]
                        .rearrange("(k p) c -> p k c", p=128))
                ops = {}

                def mm(rt, n, ks):
                    if n == 0 and ks[0] == 0:
                        ops[rt] = ps.tile([128, 2 * SCHUNK], F32,
                                          name=f"op_{rep}_{b}_{rt}",
                                          tag="pair", bufs=3)
                    if "opmm" in abl:
                        if n == 0 and ks[0] == 0:
                            nc.vector.memset(ops[rt][:, 0:8], 0.0)
                        return
                    for k in ks:
                        nc.tensor.matmul(
                            ops[rt][:, n * SCHUNK:(n + 1) * SCHUNK],
                            gsb[:, RPB * k + TT * rt:RPB * k + TT * (rt + 1)],
                            wo_sb[:, D * k + SCHUNK * n:
                                  D * k + SCHUNK * (n + 1)],
                            start=(k == 0), stop=(k == KT - 1))

                def fin(rt):
                    # (PSUM->DRAM DMA is not supported; stage through SBUF)
                    osb = outsbp.tile([128, D], F32, name=f"osb_{rep}_{b}_{rt}",
                                      tag="osb")
                    nc.vector.tensor_copy(osb[:], ops.pop(rt)[:])
                    nc.sync.dma_start(
                        out_d[(2 * b + rt) * TT:(2 * b + rt + 1) * TT, :],
                        osb[:])

                parts = []
                for rt in range(2):
                    for n in range(2):
                        parts.append((4, lambda rt=rt, n=n: mm(rt, n, [0, 1, 2, 3])))
                        parts.append((4, lambda rt=rt, n=n: mm(rt, n, [4, 5, 6, 7])))
                    parts.append((0, lambda rt=rt: fin(rt)))
                return parts

            # Software pipeline, one FLAT stream of steps across all repeats
            # (no rep-boundary special cases). Step s: attention for global
            # chunk s-1, WOVEN with the projection parts for chunk s+1 (two
            # steps of slack for the normalize chain's DVE/ACT latency) and
            # any output-projection parts scheduled for this step; x for
            # chunk s+2 is prefetched. A batch's AllToAll is issued as soon
            # as its last chunk's attention has staged its output, and its
            # output projection weaves in two steps later so the in-order PE
            # queue never head-of-line blocks on the exchange.
            chunks = [(b, i) for b in range(B) for i in range(NCH)]
            nch = len(chunks)
            total = repeat * nch
            pending = {}
            pf = [0]

            def pf_to(limit):
                while pf[0] < min(limit, total):
                    rep_box[0] = pf[0] // nch
                    prefetch_x(*chunks[pf[0] % nch])
                    pf[0] += 1

            # preamble DMA order: wq (issued at tile creation above) ->
            # x chunk 0 -> wk, wv -> consts; the first projection can then
            # start as soon as wq + the first x k-tiles land (~2us).
            pf_to(1)
            load_w("k")
            load_w("v")
            load_consts()

            for s in range(total + 1):
                pf_to(s + 3)
                if s == 1:
                    load_wo()
                weave = []
                if s + 1 < total:
                    rep_box[0] = (s + 1) // nch
                    weave += proj_parts(*chunks[(s + 1) % nch])
                weave += pending.pop(s, [])
                if s >= 1 and stage >= 2:
                    g = s - 1
                    rep_box[0] = g // nch
                    b_prev, i_prev = chunks[g % nch]
                    do_attn(b_prev, i_prev, weave=weave)
                    if stage >= 3 and i_prev == NCH - 1:
                        # this batch's last chunk is staged: exchange now;
                        # its output projection weaves in two steps later
                        # (batch 0's entirely overlaps batch 1's attention)
                        do_a2a(b_prev)
                        pending[s + 2] = outproj_parts(b_prev)
                else:
                    rep_box[0] = 0
                    do_proj(*chunks[0])
                    for _, part in weave:
                        part()
            for parts in pending.values():
                for _, part in parts:
                    part()

    nc.compile()
    return nc


_NC_CACHE = {}


def _get_nc():
    if "nc" not in _NC_CACHE:
        _NC_CACHE["nc"] = build_nc()
    return _NC_CACHE["nc"]


def _ktile_layout(w):
    # [K*128, C] -> [128, K*C]: free block k holds rows [128k, 128k+128)
    kt, c = w.shape[0] // 128, w.shape[1]
    return np.ascontiguousarray(
        w.reshape(kt, 128, c).transpose(1, 0, 2).reshape(128, kt * c))


def _host_inputs(x, Wq, Wk, Wv, Wo):
    xt = np.ascontiguousarray(
        np.asarray(x, dtype=np.float32).reshape(ROWS, D).T).astype(BF16)
    ident = np.eye(128, dtype=BF16)
    # causal-mask accumulate: out[m,n] += negtri[n,m] = -1000 iff key m >
    # query n (strictly upper triangle); exp then underflows to exactly 0
    negtri = np.triu(np.full((128, 128), -1000.0, dtype=np.float32), 1).astype(BF16)
    # block-diagonal mean-of-squares selector: selb[d, m] = 1/DH when d and
    # m fall in the same 64-row head group (output = per-head mean already
    # broadcast over the head's rows); selbq additionally folds the
    # 1/sqrt(DH) score scale into the q-half rstd (entries 1.0 = DH/DH)
    selb = np.zeros((128, 128), dtype=np.float32)
    for h in range(HEADS_PER_CORE):
        selb[h * DH:(h + 1) * DH, h * DH:(h + 1) * DH] = 1.0 / DH
    selbq = (selb * DH).astype(BF16)
    selb = selb.astype(BF16)
    wo_h = _ktile_layout(np.asarray(Wo, dtype=np.float32)).astype(BF16)

    in_maps = []
    for c in range(N_CORES):
        cs = c * DC
        in_maps.append({
            "xt": xt,
            "wq": _ktile_layout(np.asarray(Wq, dtype=np.float32)[:, cs:cs + DC]).astype(BF16),
            "wk": _ktile_layout(np.asarray(Wk, dtype=np.float32)[:, cs:cs + DC]).astype(BF16),
            "wv": _ktile_layout(np.asarray(Wv, dtype=np.float32)[:, cs:cs + DC]).astype(BF16),
            "wo": wo_h,
            "negtri": negtri,
            "ident": ident,
            "selb": selb,
            "selbq": selbq,
        })
    return in_maps


def kernel(x, Wq, Wk, Wv, Wo, mask):
    x = np.asarray(x, dtype=np.float32)
    nc = _get_nc()
    in_maps = _host_inputs(x, np.asarray(Wq), np.asarray(Wk),
                           np.asarray(Wv), np.asarray(Wo))
    res = run_bass_kernel_spmd(nc, in_maps, list(range(N_CORES)))
    full = np.empty((ROWS, D), dtype=np.float32)
    for c in range(N_CORES):
        o = res.results[c]["out"]
        for b in range(B):
            r0 = b * S + c * RPB
            full[r0:r0 + RPB] = o[b * RPB:(b + 1) * RPB]
    return full.reshape(B, S, D)


if __name__ == "__main__":
    nc = build_nc()
    print("kernel built and compiled OK")

